# revision 1
# baseline (speedup 1.0000x reference)
"""Causal self-attention (B=4, T=2048, C=768, H=12) on 8 TRN2 NeuronCores.

Sharding: DP=4 over batch x TP=2 over heads (6 heads per core).
Each core computes, for its batch b and head group g:
    kqv^T projection -> K^T,Q^T per head pair [128, T]; V natural [T, 6, 65]
    (V gets a ones column so the P~@V' matmul also yields the softmax
    denominator l as row 64.)
    S^T = K^T' Q (scores transposed, k on partitions), no max subtraction
    (scores ~ N(0,1), exp is safe in fp32), causal via a -30000 triangular
    mask add + ragged matmul/exp spans.
    O~^T' = V'^T @ exp(S^T/8) accumulated over k tiles; row 64 = l.
    Normalize: r = 1/l broadcast across partitions (gpsimd), O^T = O~^T * R.
    Y_partial = O^T_stack^T @ W_proj[rows of local heads]  -> [T, 768]
Host sums the two TP partials per batch and adds b_proj.

Matmul inputs are bf16 (TensorEngine full rate); accumulation, softmax and
normalization stay fp32.
"""

import sys

sys.path.insert(0, "/opt/trn_rl_repo")

from contextlib import ExitStack

import numpy as np

import concourse.bass as bass
import concourse.tile as tile
from concourse import bacc
from concourse import mybir
from concourse.bass import ts
from concourse.bass_utils import run_bass_kernel_spmd
from concourse.masks import make_identity

F32 = mybir.dt.float32
BF16 = mybir.dt.bfloat16

B, T, C = 4, 2048, 768
H, D = 12, 64
HL = 6          # heads per core
FL = HL * D     # 384 local feature dim
NCT = C // 128  # 6 contraction tiles
NT = T // 128   # 16 token tiles
NB = T // 512   # 4 n-blocks
NPAIR = HL // 2  # 3 head pairs

MASK_NEG = -30000.0


def build_nc():
    nc = bacc.Bacc()
    x_d = nc.declare_dram_parameter("x", [T, C], F32, isOutput=False)
    wk_d = nc.declare_dram_parameter("wk", [C, FL], F32, isOutput=False)
    wq_d = nc.declare_dram_parameter("wq", [C, FL], F32, isOutput=False)
    wv_d = nc.declare_dram_parameter("wv", [C, FL], F32, isOutput=False)
    wp_d = nc.declare_dram_parameter("wp", [FL, C], F32, isOutput=False)
    bk_d = nc.declare_dram_parameter("bk", [FL], F32, isOutput=False)
    bq_d = nc.declare_dram_parameter("bq", [FL], F32, isOutput=False)
    bv_d = nc.declare_dram_parameter("bv", [FL], F32, isOutput=False)
    mask_d = nc.declare_dram_parameter("mask", [128, 128], F32, isOutput=False)
    y_d = nc.declare_dram_parameter("y", [T, C], F32, isOutput=True)

    with tile.TileContext(nc) as tc, ExitStack() as ctx:
        const = ctx.enter_context(tc.tile_pool(name="const", bufs=1))
        wpool = ctx.enter_context(tc.tile_pool(name="wpool", bufs=1))
        big = ctx.enter_context(tc.tile_pool(name="big", bufs=1))
        xtp = ctx.enter_context(tc.tile_pool(name="xtp", bufs=8))
        xin = ctx.enter_context(tc.tile_pool(name="xin", bufs=2))
        ppool = ctx.enter_context(tc.tile_pool(name="ppool", bufs=4))
        small = ctx.enter_context(tc.tile_pool(name="small", bufs=2))
        ypool = ctx.enter_context(tc.tile_pool(name="ypool", bufs=2))
        mmps = ctx.enter_context(tc.tile_pool(name="mmps", bufs=4, space="PSUM"))
        spool = ctx.enter_context(tc.tile_pool(name="spool", bufs=2, space="PSUM"))

        # ---- constants ----
        ident = const.tile([128, 128], BF16)
        make_identity(nc, ident)
        # mask[k, q] = 0 where k <= q (causal-valid), else MASK_NEG
        # (bf16: accumulated into scores by a PE matmul with identity lhsT)
        trimask = const.tile([128, 128], BF16)
        nc.gpsimd.dma_start(out=trimask, in_=mask_d[:, :])
        ones_sb = const.tile([1, 128], BF16)
        nc.vector.memset(ones_sb, 1.0)
        bk_sb = const.tile([128, NPAIR], F32)
        bq_sb = const.tile([128, NPAIR], F32)
        nc.gpsimd.dma_start(out=bk_sb, in_=bk_d.rearrange("(i p) -> p i", p=128))
        nc.gpsimd.dma_start(out=bq_sb, in_=bq_d.rearrange("(i p) -> p i", p=128))
        # gpsimd (SWDGE) DMAs cast fp32 DRAM -> bf16 SBUF in flight
        bv_sb = const.tile([1, FL], BF16)
        nc.gpsimd.dma_start(out=bv_sb, in_=bv_d.rearrange("(o f) -> o f", o=1))

        # ---- weights: cast-DMA straight to bf16 ----
        wk_t, wq_t, wv_t, wp_t = [], [], [], []
        for ct in range(NCT):
            wkt = wpool.tile([128, FL], BF16, tag="wk", bufs=NCT, name=f"wk{ct}")
            wqt = wpool.tile([128, FL], BF16, tag="wq", bufs=NCT, name=f"wq{ct}")
            wvt = wpool.tile([128, FL], BF16, tag="wv", bufs=NCT, name=f"wv{ct}")
            nc.gpsimd.dma_start(out=wkt, in_=wk_d[ts(ct, 128), :])
            nc.gpsimd.dma_start(out=wqt, in_=wq_d[ts(ct, 128), :])
            nc.gpsimd.dma_start(out=wvt, in_=wv_d[ts(ct, 128), :])
            wk_t.append(wkt)
            wq_t.append(wqt)
            wv_t.append(wvt)
        for i in range(NPAIR):
            wpt = wpool.tile([128, C], BF16, tag="wp", bufs=NPAIR, name=f"wp{i}")
            nc.gpsimd.dma_start(out=wpt, in_=wp_d[ts(i, 128), :])
            wp_t.append(wpt)

        # ---- persistent activations ----
        kt_sb = [
            big.tile([128, T], BF16, tag="ktq", bufs=2 * NPAIR, name=f"ktp{i}")
            for i in range(NPAIR)
        ]
        qt_sb = [
            big.tile([128, T], BF16, tag="ktq", bufs=2 * NPAIR, name=f"qtp{i}")
            for i in range(NPAIR)
        ]
        v_sb = [
            big.tile([128, HL, D + 1], BF16, tag="v", bufs=NT, name=f"v{t}")
            for t in range(NT)
        ]
        otn_sb = [
            big.tile([128, T], BF16, tag="otn", bufs=NPAIR, name=f"otn{i}")
            for i in range(NPAIR)
        ]

        # ---- phase 1+2: X^T tiles, kqv^T projections, V natural ----
        for n in range(NB):
            xt_n = [
                xtp.tile([128, 512], BF16, tag="xt", name=f"xt{ct}_{n}")
                for ct in range(NCT)
            ]
            for tt in range(4):
                t = 4 * n + tt
                xb_sb = xin.tile([128, C], BF16, tag="xb", bufs=NT, name=f"xb{t}")
                nc.gpsimd.dma_start(out=xb_sb, in_=x_d[ts(t, 128), :])
                for ct in range(NCT):
                    nc.sync.dma_start_transpose(
                        out=xt_n[ct][:, ts(tt, 128)],
                        in_=xb_sb[:, ts(ct, 128)],
                    )
            # kqv^T: K^T and Q^T pair tiles [128, T]
            for m in range(2 * NPAIR):
                w_src = wk_t if m < NPAIR else wq_t
                mi = m % NPAIR
                ps = mmps.tile([128, 512], F32, tag="mm", name=f"kqps{n}_{m}")
                for ct in range(NCT):
                    nc.tensor.matmul(
                        out=ps,
                        lhsT=w_src[ct][:, ts(mi, 128)],
                        rhs=xt_n[ct],
                        start=(ct == 0),
                        stop=(ct == NCT - 1),
                    )
                dest = kt_sb[mi] if m < NPAIR else qt_sb[mi]
                bias = (bk_sb if m < NPAIR else bq_sb)[:, mi : mi + 1]
                nc.vector.tensor_scalar_add(
                    out=dest[:, ts(n, 512)], in0=ps, scalar1=bias
                )
            # V natural (+bias via ones-row K=1 matmul)
            for tt in range(4):
                t = 4 * n + tt
                psv = mmps.tile([128, FL], F32, tag="mm", name=f"vps{t}")
                for ct in range(NCT):
                    nc.tensor.matmul(
                        out=psv,
                        lhsT=xt_n[ct][:, ts(tt, 128)],
                        rhs=wv_t[ct],
                        start=(ct == 0),
                        stop=False,
                    )
                nc.tensor.matmul(
                    out=psv,
                    lhsT=ones_sb,
                    rhs=bv_sb,
                    start=False,
                    stop=True,
                )
                nc.vector.tensor_copy(
                    out=v_sb[t][:, :, 0:D],
                    in_=psv.rearrange("p (h d) -> p h d", h=HL),
                )
                nc.gpsimd.memset(v_sb[t][:, :, D : D + 1], 1.0)

        # ---- phase 3: attention, two heads of a pair interleaved ----
        # Each (pair, J) works on a 1024-wide query half; the two heads get
        # independent S / OT psum tiles so the scheduler can run one head's
        # matmuls while the other head's exp is on the scalar engine.
        for pair in range(NPAIR):
            for J in range(2):
                hs = (2 * pair, 2 * pair + 1)
                q0 = 1024 * J
                sps_h = {}
                ot_h = {}
                pb_h = {}
                for h in hs:
                    sps_h[h] = spool.tile(
                        [128, 1024], F32, tag="s", name=f"s{h}_{J}"
                    )
                    ot_h[h] = {
                        j: mmps.tile(
                            [128, 512], F32, tag="mm", name=f"ot{h}_{j}"
                        )
                        for j in (2 * J, 2 * J + 1)
                    }
                for kt in range(8 * J + 8):
                    c0 = 128 * kt
                    diag = 8 * J <= kt  # diagonal block lands in this half
                    for h in hs:
                        row0 = 64 * (h % 2)
                        kt_ap = kt_sb[pair][row0 : row0 + 64, :]
                        qt_ap = qt_sb[pair][row0 : row0 + 64, :]
                        sps = sps_h[h]
                        for j in range(max(2 * J, kt // 4), 2 * J + 2):
                            s0 = max(512 * j, c0)
                            w = 512 * (j + 1) - s0
                            nc.tensor.matmul(
                                out=sps[:, s0 - q0 : s0 - q0 + w],
                                lhsT=kt_ap[:, ts(kt, 128)],
                                rhs=qt_ap[:, s0 : s0 + w],
                                start=True,
                                stop=True,
                                tile_position=(row0, 0),
                                skip_group_check=True,
                            )
                        if diag:
                            # causal mask on the diagonal 128x128 block
                            nc.vector.tensor_add(
                                sps[:, c0 - q0 : c0 - q0 + 128],
                                sps[:, c0 - q0 : c0 - q0 + 128],
                                trimask,
                            )
                        e0 = max(q0, c0)
                        pb = ppool.tile(
                            [128, 1024], BF16, tag="p", name=f"p{h}_{kt}_{J}"
                        )
                        nc.scalar.activation(
                            out=pb[:, e0 - q0 : 1024],
                            in_=sps[:, e0 - q0 : 1024],
                            func=mybir.ActivationFunctionType.Exp,
                            scale=float(D) ** -0.5,
                        )
                        if diag and kt % 4:
                            # stale prefix of the diagonal 512-block -> 0
                            nc.gpsimd.memset(
                                pb[:, 512 * (kt // 4) - q0 : c0 - q0], 0.0
                            )
                        for j in range(max(2 * J, kt // 4), 2 * J + 2):
                            s0 = max(512 * j, c0)
                            w = 512 * (j + 1) - s0
                            nc.tensor.matmul(
                                out=ot_h[h][j][
                                    0 : D + 1, s0 - 512 * j : s0 - 512 * j + w
                                ],
                                lhsT=v_sb[kt][:, h, :],
                                rhs=pb[:, s0 - q0 : s0 - q0 + w],
                                start=(kt == 0),
                                stop=(kt == 4 * j + 3),
                            )
                for h in hs:
                    row0 = 64 * (h % 2)
                    for j in (2 * J, 2 * J + 1):
                        otps = ot_h[h][j]
                        lv = small.tile([1, 512], F32, tag="l", name=f"l{h}_{j}")
                        nc.scalar.copy(out=lv, in_=otps[D : D + 1, :])
                        rv = small.tile([1, 512], F32, tag="r", name=f"r{h}_{j}")
                        nc.vector.reciprocal_approx_fast(out=rv, in_=lv)
                        rb = small.tile([64, 512], F32, tag="R", name=f"R{h}_{j}")
                        nc.gpsimd.partition_broadcast(rb, rv)
                        nc.vector.tensor_mul(
                            otn_sb[pair][row0 : row0 + 64, ts(j, 512)],
                            otps[0:D, :],
                            rb,
                        )

        # ---- phase 4: output projection (partial; host adds TP pair + bias) ----
        for qi in range(NT):
            y_sb = ypool.tile([128, C], F32, tag="y", bufs=NT, name=f"y{qi}")
            for half in range(2):
                fps = mmps.tile([128, FL], F32, tag="mm", name=f"fps{qi}_{half}")
                for pair in range(NPAIR):
                    nc.tensor.matmul(
                        out=fps,
                        lhsT=otn_sb[pair][:, ts(qi, 128)],
                        rhs=wp_t[pair][:, ts(half, FL)],
                        start=(pair == 0),
                        stop=(pair == NPAIR - 1),
                    )
                nc.vector.tensor_copy(out=y_sb[:, ts(half, FL)], in_=fps)
            nc.gpsimd.dma_start(out=y_d[ts(qi, 128), :], in_=y_sb)

    nc.compile()
    return nc


_NC = None


def _get_nc():
    global _NC
    if _NC is None:
        _NC = build_nc()
    return _NC


def make_in_maps(x, W_kqv, b_kqv, W_proj):
    ki = np.arange(128)[:, None]
    qi = np.arange(128)[None, :]
    mask = np.where(ki <= qi, 0.0, MASK_NEG).astype(np.float32)
    in_maps = []
    for core in range(8):
        b = core // 2
        h0 = (core % 2) * HL * D  # feature offset of this core's head group
        in_maps.append(
            {
                "x": np.ascontiguousarray(x[b]),
                "wk": np.ascontiguousarray(W_kqv[:, h0 : h0 + FL]),
                "wq": np.ascontiguousarray(W_kqv[:, C + h0 : C + h0 + FL]),
                "wv": np.ascontiguousarray(W_kqv[:, 2 * C + h0 : 2 * C + h0 + FL]),
                "wp": np.ascontiguousarray(W_proj[h0 : h0 + FL, :]),
                "bk": np.ascontiguousarray(b_kqv[h0 : h0 + FL]),
                "bq": np.ascontiguousarray(b_kqv[C + h0 : C + h0 + FL]),
                "bv": np.ascontiguousarray(b_kqv[2 * C + h0 : 2 * C + h0 + FL]),
                "mask": mask,
            }
        )
    return in_maps


def _combine(results, b_proj):
    y = np.empty((B, T, C), dtype=np.float32)
    for b in range(B):
        y[b] = results[2 * b]["y"] + results[2 * b + 1]["y"] + b_proj[None, :]
    return y


def kernel(x, W_kqv, b_kqv, W_proj, b_proj, **run_kwargs):
    x = np.asarray(x, dtype=np.float32)
    W_kqv = np.asarray(W_kqv, dtype=np.float32)
    b_kqv = np.asarray(b_kqv, dtype=np.float32)
    W_proj = np.asarray(W_proj, dtype=np.float32)
    b_proj = np.asarray(b_proj, dtype=np.float32)

    nc = _get_nc()
    in_maps = make_in_maps(x, W_kqv, b_kqv, W_proj)
    res = run_bass_kernel_spmd(nc, in_maps, core_ids=list(range(8)), **run_kwargs)
    out = _combine(res.results, b_proj)
    kernel.last_result = res
    return out



# revision 9
# speedup vs baseline: 1.7492x; 1.7492x over previous
"""Causal self-attention (B=4, T=2048, C=768, H=12) on 8 TRN2 NeuronCores.

Sharding: DP=4 over batch x TP=2 over heads (6 heads per core).

Single software-pipelined stream per core, ordered so the PE never idles
(keeps the HAM clock-gate warm at 2.4 GHz):

  n-block 0 kqv -> chunk0 attn (+nb1 kqv filler) -> chunk1 attn (+nb2)
  -> chunk2 attn (+nb3) -> chunk3 attn (+proj filler) -> proj tail

Per n-block (512 tokens): X tiles cast-DMA'd to bf16, transposed on the
PE (identity matmul, 4 per PSUM bank, one DVE copy out), then K^T/Q^T
pair tiles [128, T] (bias via DVE tensor_scalar_add) and V natural
[128, (h, 65)] with a ones column so P~@V' also yields the softmax
denominator l as row 64.

Attention per 512-query chunk: two head-chains in flight; S^T for TWO
k-tiles side by side in a [128,1024] PSUM (ragged causal spans), causal
mask added on the diagonal blocks by an extra PE matmul (identity x
trimask accumulate), one exp (ACT, scale=1/8, fp32->bf16) per k-tile
pair, then P~@V' accumulation. Normalize: r = 1/l via DVE reciprocal
straight off the PSUM l-row, gpsimd partition-broadcast, DVE multiply.

Output projection per 128-token tile interleaved as filler; host sums
the two TP partials per batch and adds b_proj.
"""

import sys

sys.path.insert(0, "/opt/trn_rl_repo")

from collections import deque
from contextlib import ExitStack

import numpy as np

import concourse.bass as bass
import concourse.tile as tile
from concourse import bacc
from concourse import mybir
from concourse.bass import ts
from concourse.bass_utils import run_bass_kernel_spmd
from concourse.masks import make_identity

F32 = mybir.dt.float32
BF16 = mybir.dt.bfloat16

B, T, C = 4, 2048, 768
H, D = 12, 64
HL = 6          # heads per core
FL = HL * D     # 384 local feature dim
NCT = C // 128  # 6 contraction tiles
NT = T // 128   # 16 token tiles
NB = T // 512   # 4 n-blocks / query chunks
NP = HL // 2    # 3 head pairs

MASK_NEG = -30000.0

DEBUG = False  # set True (before build) to add intermediate dumps


def build_nc():
    nc = bacc.Bacc()
    x_d = nc.declare_dram_parameter("x", [T, C], F32, isOutput=False)
    wk_d = nc.declare_dram_parameter("wk", [C, FL], F32, isOutput=False)
    wq_d = nc.declare_dram_parameter("wq", [C, FL], F32, isOutput=False)
    wv_d = nc.declare_dram_parameter("wv", [C, FL], F32, isOutput=False)
    wp_d = nc.declare_dram_parameter("wp", [FL, C], F32, isOutput=False)
    bk_d = nc.declare_dram_parameter("bk", [FL], F32, isOutput=False)
    bq_d = nc.declare_dram_parameter("bq", [FL], F32, isOutput=False)
    bv_d = nc.declare_dram_parameter("bv", [FL], F32, isOutput=False)
    mask_d = nc.declare_dram_parameter("mask", [128, 128], F32, isOutput=False)
    y_d = nc.declare_dram_parameter("y", [T, C], F32, isOutput=True)
    if DEBUG:
        dbg_v = nc.declare_dram_parameter(
            "dbg_v", [128, NT * HL * (D + 1)], F32, isOutput=True
        )
        dbg_kt = nc.declare_dram_parameter("dbg_kt", [128, T], F32, isOutput=True)
        dbg_qt = nc.declare_dram_parameter("dbg_qt", [128, T], F32, isOutput=True)
        dbg_otn = nc.declare_dram_parameter("dbg_otn", [128, T], F32, isOutput=True)
        dbg_l = nc.declare_dram_parameter("dbg_l", [2, T], F32, isOutput=True)

    with tile.TileContext(nc) as tc, ExitStack() as ctx:
        const = ctx.enter_context(tc.tile_pool(name="const", bufs=1))
        wpool = ctx.enter_context(tc.tile_pool(name="wpool", bufs=1))
        big = ctx.enter_context(tc.tile_pool(name="big", bufs=1))
        xbp = ctx.enter_context(tc.tile_pool(name="xbp", bufs=8))
        xtp = ctx.enter_context(tc.tile_pool(name="xtp", bufs=2))
        ppool = ctx.enter_context(tc.tile_pool(name="ppool", bufs=4))
        smal = ctx.enter_context(tc.tile_pool(name="smal", bufs=2))
        ypool = ctx.enter_context(tc.tile_pool(name="ypool", bufs=2))
        spool = ctx.enter_context(tc.tile_pool(name="spool", bufs=2, space="PSUM"))
        otps = ctx.enter_context(tc.tile_pool(name="otps", bufs=2, space="PSUM"))
        gpp = ctx.enter_context(tc.tile_pool(name="gpp", bufs=2, space="PSUM"))

        # ---- constants ----
        ident = const.tile([128, 128], BF16)
        make_identity(nc, ident)
        # trimask[k, q] = 0 where k <= q (causal-valid), else MASK_NEG
        trimask = const.tile([128, 128], BF16)
        nc.gpsimd.dma_start(out=trimask, in_=mask_d[:, :])
        ones_sb = const.tile([1, 128], BF16)
        nc.vector.memset(ones_sb, 1.0)
        bk_sb = const.tile([128, NP], F32)
        bq_sb = const.tile([128, NP], F32)
        nc.gpsimd.dma_start(out=bk_sb, in_=bk_d.rearrange("(i p) -> p i", p=128))
        nc.gpsimd.dma_start(out=bq_sb, in_=bq_d.rearrange("(i p) -> p i", p=128))
        bv_sb = const.tile([1, FL], BF16)
        nc.gpsimd.dma_start(out=bv_sb, in_=bv_d.rearrange("(o f) -> o f", o=1))

        # ---- weights: cast-DMA straight to bf16 ----
        wk_t, wq_t, wv_t, wp_t = [], [], [], []
        for ct in range(NCT):
            wkt = wpool.tile([128, FL], BF16, tag="wk", bufs=NCT, name=f"wk{ct}")
            wqt = wpool.tile([128, FL], BF16, tag="wq", bufs=NCT, name=f"wq{ct}")
            wvt = wpool.tile([128, FL], BF16, tag="wv", bufs=NCT, name=f"wv{ct}")
            nc.gpsimd.dma_start(out=wkt, in_=wk_d[ts(ct, 128), :])
            nc.gpsimd.dma_start(out=wqt, in_=wq_d[ts(ct, 128), :])
            nc.gpsimd.dma_start(out=wvt, in_=wv_d[ts(ct, 128), :])
            wk_t.append(wkt)
            wq_t.append(wqt)
            wv_t.append(wvt)
        for p in range(NP):
            wpt = wpool.tile([128, C], BF16, tag="wp", bufs=NP, name=f"wp{p}")
            nc.gpsimd.dma_start(out=wpt, in_=wp_d[ts(p, 128), :])
            wp_t.append(wpt)

        # ---- persistent activations ----
        kt_sb = [
            big.tile([128, T], BF16, tag="ktq", bufs=2 * NP, name=f"ktp{p}")
            for p in range(NP)
        ]
        qt_sb = [
            big.tile([128, T], BF16, tag="ktq", bufs=2 * NP, name=f"qtp{p}")
            for p in range(NP)
        ]
        v_all = big.tile([128, NT, HL, D + 1], BF16, tag="v", bufs=1)
        nc.gpsimd.memset(v_all[:, :, :, D : D + 1], 1.0)
        otn = [
            big.tile([128, T], BF16, tag="otn", bufs=NP, name=f"otn{p}")
            for p in range(NP)
        ]

        # ---- filler queue: closures emitted into PE-stall slots ----
        filler = deque()

        def pump(n):
            for _ in range(n):
                if not filler:
                    return
                filler.popleft()()

        def pump_all():
            while filler:
                filler.popleft()()

        # ---- per-n-block projection work ----
        xb_tiles = {}

        def emit_x_dma(n):
            for tt in range(4):
                t = 4 * n + tt
                xb = xbp.tile([128, C], BF16, tag="xb", name=f"xb{t}")
                nc.gpsimd.dma_start(out=xb, in_=x_d[ts(t, 128), :])
                xb_tiles[t] = xb

        def nb_closures(n):
            xt = xtp.tile([128, NCT, 512], BF16, tag="xt", name=f"xt{n}")
            items = []

            def tr(ct):
                psT = gpp.tile([128, 512], BF16, tag="gp", name=f"trp{n}_{ct}")
                for tt in range(4):
                    nc.tensor.transpose(
                        psT[:, ts(tt, 128)],
                        xb_tiles[4 * n + tt][:, ts(ct, 128)],
                        ident,
                    )
                nc.vector.tensor_copy(out=xt[:, ct, :], in_=psT)

            for ct in range(NCT):
                items.append(lambda ct=ct: tr(ct))

            def kq(m):
                src = wk_t if m < NP else wq_t
                mi = m % NP
                dest = kt_sb[mi] if m < NP else qt_sb[mi]
                bias = (bk_sb if m < NP else bq_sb)[:, mi : mi + 1]
                ps = gpp.tile([128, 512], F32, tag="gp", name=f"kqp{n}_{m}")
                for ct in range(NCT):
                    nc.tensor.matmul(
                        out=ps,
                        lhsT=src[ct][:, ts(mi, 128)],
                        rhs=xt[:, ct, :],
                        start=(ct == 0),
                        stop=(ct == NCT - 1),
                    )
                nc.vector.tensor_scalar_add(
                    out=dest[:, ts(n, 512)], in0=ps, scalar1=bias
                )

            for m in range(2 * NP):
                items.append(lambda m=m: kq(m))

            def vv(tt):
                t = 4 * n + tt
                ps = gpp.tile([128, 512], F32, tag="gp", name=f"vp{t}")
                for ct in range(NCT):
                    nc.tensor.matmul(
                        out=ps[:, 0:FL],
                        lhsT=xt[:, ct, ts(tt, 128)],
                        rhs=wv_t[ct],
                        start=(ct == 0),
                        stop=False,
                    )
                nc.tensor.matmul(
                    out=ps[:, 0:FL],
                    lhsT=ones_sb,
                    rhs=bv_sb,
                    start=False,
                    stop=True,
                )
                nc.vector.tensor_copy(
                    out=v_all[:, t, :, 0:D],
                    in_=ps[:, 0:FL].rearrange("p (h d) -> p h d", h=HL),
                )

            for tt in range(4):
                items.append(lambda tt=tt: vv(tt))
            return items

        # ---- projection (filler) ----
        def proj_closures(qi):
            y_sb = ypool.tile([128, C], F32, tag="y", name=f"y{qi}")
            items = []

            def half(hf):
                ps = gpp.tile([128, 512], F32, tag="gp", name=f"fp{qi}_{hf}")
                for p in range(NP):
                    nc.tensor.matmul(
                        out=ps[:, 0:FL],
                        lhsT=otn[p][:, ts(qi, 128)],
                        rhs=wp_t[p][:, ts(hf, FL)],
                        start=(p == 0),
                        stop=(p == NP - 1),
                    )
                if hf == 0:
                    nc.vector.tensor_copy(out=y_sb[:, ts(hf, FL)], in_=ps[:, 0:FL])
                else:
                    nc.scalar.copy(out=y_sb[:, ts(hf, FL)], in_=ps[:, 0:FL])

            items.append(lambda: half(0))
            items.append(lambda: half(1))
            items.append(
                lambda: nc.sync.dma_start(out=y_d[ts(qi, 128), :], in_=y_sb)
            )
            return items

        # ---- attention chunk j (512 queries, all heads) ----
        def emit_chunk(j):
            q0 = 512 * j
            niter = 2 * j + 2  # k-tile pairs
            for p in range(NP):
                hs = (2 * p, 2 * p + 1)
                S = {}
                OT = {}
                for h in hs:
                    S[h] = spool.tile(
                        [128, 1024], F32, tag="s", name=f"s{h}_{j}"
                    )
                    OT[h] = otps.tile(
                        [128, 512], F32, tag="ot", name=f"ot{h}_{j}"
                    )
                for i in range(niter):
                    kts = (2 * i, 2 * i + 1)
                    sA = max(0, 128 * kts[0] - q0)
                    for h in hs:
                        row0 = 64 * (h % 2)
                        kt_ap = kt_sb[p][row0 : row0 + 64, :]
                        qt_ap = qt_sb[p][row0 : row0 + 64, :]
                        for idx, kt in enumerate(kts):
                            c0 = 128 * kt
                            diag = c0 >= q0
                            s0 = max(q0, c0)
                            w = q0 + 512 - s0
                            o = 512 * idx + s0 - q0
                            nc.tensor.matmul(
                                out=S[h][:, o : o + w],
                                lhsT=kt_ap[:, ts(kt, 128)],
                                rhs=qt_ap[:, s0 : s0 + w],
                                start=True,
                                stop=not diag,
                                tile_position=(row0, 0),
                                skip_group_check=True,
                            )
                            if diag:
                                nc.tensor.matmul(
                                    out=S[h][:, o : o + 128],
                                    lhsT=ident,
                                    rhs=trimask,
                                    start=False,
                                    stop=True,
                                    skip_group_check=True,
                                )
                    pump(1)
                    pbs = {}
                    for h in hs:
                        pb = ppool.tile(
                            [128, 1024], BF16, tag="pb", name=f"pb{h}_{i}_{j}"
                        )
                        nc.scalar.activation(
                            out=pb[:, sA:1024],
                            in_=S[h][:, sA:1024],
                            func=mybir.ActivationFunctionType.Exp,
                            scale=float(D) ** -0.5,
                        )
                        pbs[h] = pb
                    for h in hs:
                        pb = pbs[h]
                        for idx, kt in enumerate(kts):
                            s = max(0, 128 * kt - q0)
                            nc.tensor.matmul(
                                out=OT[h][0 : D + 1, s:512],
                                lhsT=v_all[:, kt, h, :],
                                rhs=pb[:, 512 * idx + s : 512 * idx + 512],
                                start=(i == 0 and idx == 0),
                                stop=(i == niter - 1 and idx == 1),
                                skip_group_check=True,
                            )
                    pump(1)
                # normalize: r = 1/l broadcast over partitions
                for h in hs:
                    row0 = 64 * (h % 2)
                    lv = smal.tile([1, 512], F32, tag="lv", name=f"lv{h}_{j}")
                    nc.vector.tensor_copy(out=lv, in_=OT[h][D : D + 1, :])
                    rv = smal.tile([1, 512], F32, tag="rv", name=f"rv{h}_{j}")
                    nc.vector.reciprocal_approx_fast(out=rv, in_=lv)
                    rb = smal.tile([64, 512], F32, tag="rb", name=f"rb{h}_{j}")
                    nc.gpsimd.partition_broadcast(rb, rv)
                    nc.vector.tensor_mul(
                        otn[p][row0 : row0 + 64, ts(j, 512)], OT[h][0:D, :], rb
                    )
                    if DEBUG and p == 0:
                        nc.gpsimd.dma_start(
                            out=dbg_l[h : h + 1, ts(j, 512)], in_=lv
                        )
                pump(1)

        # ---- main schedule ----
        emit_x_dma(0)
        for it in nb_closures(0):
            it()
        for j in range(NB):
            if j + 1 < NB:
                emit_x_dma(j + 1)
                filler.extend(nb_closures(j + 1))
            else:
                for qi in range(12):
                    filler.extend(proj_closures(qi))
            emit_chunk(j)
            pump_all()
        for qi in range(12, NT):
            for it in proj_closures(qi):
                it()
        if DEBUG:
            nc.gpsimd.dma_start(
                out=dbg_v[:, :], in_=v_all.rearrange("p a b c -> p (a b c)")
            )
            nc.gpsimd.dma_start(out=dbg_kt[:, :], in_=kt_sb[0])
            nc.gpsimd.dma_start(out=dbg_qt[:, :], in_=qt_sb[0])
            nc.gpsimd.dma_start(out=dbg_otn[:, :], in_=otn[0])

    nc.compile()
    return nc


_NC = None


def _get_nc():
    global _NC
    if _NC is None:
        _NC = build_nc()
    return _NC


def make_in_maps(x, W_kqv, b_kqv, W_proj):
    ki = np.arange(128)[:, None]
    qi = np.arange(128)[None, :]
    mask = np.where(ki <= qi, 0.0, MASK_NEG).astype(np.float32)
    in_maps = []
    for core in range(8):
        b = core // 2
        h0 = (core % 2) * HL * D  # feature offset of this core's head group
        in_maps.append(
            {
                "x": np.ascontiguousarray(x[b]),
                "wk": np.ascontiguousarray(W_kqv[:, h0 : h0 + FL]),
                "wq": np.ascontiguousarray(W_kqv[:, C + h0 : C + h0 + FL]),
                "wv": np.ascontiguousarray(W_kqv[:, 2 * C + h0 : 2 * C + h0 + FL]),
                "wp": np.ascontiguousarray(W_proj[h0 : h0 + FL, :]),
                "bk": np.ascontiguousarray(b_kqv[h0 : h0 + FL]),
                "bq": np.ascontiguousarray(b_kqv[C + h0 : C + h0 + FL]),
                "bv": np.ascontiguousarray(b_kqv[2 * C + h0 : 2 * C + h0 + FL]),
                "mask": mask,
            }
        )
    return in_maps


def _combine(results, b_proj):
    y = np.empty((B, T, C), dtype=np.float32)
    for b in range(B):
        y[b] = results[2 * b]["y"] + results[2 * b + 1]["y"] + b_proj[None, :]
    return y


def kernel(x, W_kqv, b_kqv, W_proj, b_proj, **run_kwargs):
    x = np.asarray(x, dtype=np.float32)
    W_kqv = np.asarray(W_kqv, dtype=np.float32)
    b_kqv = np.asarray(b_kqv, dtype=np.float32)
    W_proj = np.asarray(W_proj, dtype=np.float32)
    b_proj = np.asarray(b_proj, dtype=np.float32)

    nc = _get_nc()
    in_maps = make_in_maps(x, W_kqv, b_kqv, W_proj)
    res = run_bass_kernel_spmd(nc, in_maps, core_ids=list(range(8)), **run_kwargs)
    out = _combine(res.results, b_proj)
    kernel.last_result = res
    return out


# revision 18
# speedup vs baseline: 1.8792x; 1.0744x over previous
"""Causal self-attention (B=4, T=2048, C=768, H=12) on 8 TRN2 NeuronCores.

Sharding: DP=4 over batch x TP=2 over heads (6 heads per core).

Single software-pipelined stream per core, ordered so the PE never idles
(keeps the HAM clock-gate warm at 2.4 GHz):

  n-block 0 kqv -> chunk0 attn (+nb1 kqv filler) -> chunk1 attn (+nb2)
  -> chunk2 attn (+nb3) -> chunk3 attn (+proj filler) -> proj tail

Per n-block (512 tokens): X tiles cast-DMA'd to bf16, transposed on the
PE (identity matmul, 4 per PSUM bank, one DVE copy out), then K^T/Q^T
pair tiles [128, T] (bias via DVE tensor_scalar_add) and V natural
[128, (h, 65)] with a ones column so P~@V' also yields the softmax
denominator l as row 64.

Attention per 512-query chunk: two head-chains in flight; S^T for TWO
k-tiles side by side in a [128,1024] PSUM (ragged causal spans), causal
mask added on the diagonal blocks by an extra PE matmul (identity x
trimask accumulate), one exp (ACT, scale=1/8, fp32->bf16) per k-tile
pair, then P~@V' accumulation. Normalize: r = 1/l via DVE reciprocal
straight off the PSUM l-row, gpsimd partition-broadcast, DVE multiply.

Output projection per 128-token tile interleaved as filler; host sums
the two TP partials per batch and adds b_proj.
"""

import sys

sys.path.insert(0, "/opt/trn_rl_repo")

from collections import deque
from contextlib import ExitStack

import numpy as np

import concourse.bass as bass
import concourse.tile as tile
from concourse import bacc
from concourse import mybir
from concourse.bass import ts
from concourse.bass_utils import run_bass_kernel_spmd
from concourse.masks import make_identity

F32 = mybir.dt.float32
BF16 = mybir.dt.bfloat16

B, T, C = 4, 2048, 768
H, D = 12, 64
HL = 6          # heads per core
FL = HL * D     # 384 local feature dim
NCT = C // 128  # 6 contraction tiles
NT = T // 128   # 16 token tiles
NB = T // 512   # 4 n-blocks / query chunks
NP = HL // 2    # 3 head pairs

MASK_NEG = -30000.0

DEBUG = False  # set True (before build) to add intermediate dumps


def build_nc():
    nc = bacc.Bacc()
    x_d = nc.declare_dram_parameter("x", [T, C], F32, isOutput=False)
    wk_d = nc.declare_dram_parameter("wk", [C, FL], F32, isOutput=False)
    wq_d = nc.declare_dram_parameter("wq", [C, FL], F32, isOutput=False)
    wv_d = nc.declare_dram_parameter("wv", [C, FL], F32, isOutput=False)
    wp_d = nc.declare_dram_parameter("wp", [FL, C], F32, isOutput=False)
    bk_d = nc.declare_dram_parameter("bk", [FL], F32, isOutput=False)
    bq_d = nc.declare_dram_parameter("bq", [FL], F32, isOutput=False)
    bv_d = nc.declare_dram_parameter("bv", [FL], F32, isOutput=False)
    mask_d = nc.declare_dram_parameter("mask", [128, 128], F32, isOutput=False)
    y_d = nc.declare_dram_parameter("y", [T, C], F32, isOutput=True)
    if DEBUG:
        dbg_v = nc.declare_dram_parameter(
            "dbg_v", [128, NT * HL * (D + 1)], F32, isOutput=True
        )
        dbg_kt = nc.declare_dram_parameter("dbg_kt", [128, T], F32, isOutput=True)
        dbg_qt = nc.declare_dram_parameter("dbg_qt", [128, T], F32, isOutput=True)
        dbg_otn = nc.declare_dram_parameter("dbg_otn", [128, T], F32, isOutput=True)
        dbg_l = nc.declare_dram_parameter("dbg_l", [2, T], F32, isOutput=True)

    with tile.TileContext(nc) as tc, ExitStack() as ctx:
        const = ctx.enter_context(tc.tile_pool(name="const", bufs=1))
        wpool = ctx.enter_context(tc.tile_pool(name="wpool", bufs=1))
        big = ctx.enter_context(tc.tile_pool(name="big", bufs=1))
        xbp = ctx.enter_context(tc.tile_pool(name="xbp", bufs=3))
        xtp = ctx.enter_context(tc.tile_pool(name="xtp", bufs=2))
        ppool = ctx.enter_context(tc.tile_pool(name="ppool", bufs=4))
        smal = ctx.enter_context(tc.tile_pool(name="smal", bufs=2))
        ypool = ctx.enter_context(tc.tile_pool(name="ypool", bufs=2))
        spool = ctx.enter_context(tc.tile_pool(name="spool", bufs=2, space="PSUM"))
        otps = ctx.enter_context(tc.tile_pool(name="otps", bufs=2, space="PSUM"))
        gpp = ctx.enter_context(tc.tile_pool(name="gpp", bufs=2, space="PSUM"))

        # ---- x n-block 0 first: the PE pipeline starts on it ----
        xb_tiles = {}

        def emit_x_dma(n):
            xb = xbp.tile([128, 4, C], BF16, tag="xb", name=f"xb{n}")
            nc.gpsimd.dma_start(
                out=xb,
                in_=x_d.rearrange("(n t p) c -> p n t c", n=NB, t=4)[:, n, :, :],
            )
            xb_tiles[n] = xb

        # ---- constants / weights: cast-DMA straight to bf16, one DMA each ----
        ident = const.tile([128, 128], BF16)
        make_identity(nc, ident)
        wk_all = wpool.tile([128, NCT, FL], BF16, tag="wk")
        wq_all = wpool.tile([128, NCT, FL], BF16, tag="wq")
        wv_all = wpool.tile([128, NCT, FL], BF16, tag="wv")

        def emit_w_dma():
            for w_all, w_d in ((wk_all, wk_d), (wq_all, wq_d), (wv_all, wv_d)):
                nc.gpsimd.dma_start(
                    out=w_all, in_=w_d.rearrange("(ct p) f -> p ct f", p=128)
                )

        wk_t = [wk_all[:, ct, :] for ct in range(NCT)]
        wq_t = [wq_all[:, ct, :] for ct in range(NCT)]
        wv_t = [wv_all[:, ct, :] for ct in range(NCT)]
        # trimask[k, q] = 0 where k <= q (causal-valid), else MASK_NEG
        trimask = const.tile([128, 128], BF16)
        ones_sb = const.tile([1, 128], BF16)
        nc.vector.memset(ones_sb, 1.0)
        bk_sb = const.tile([128, NP], F32)
        bq_sb = const.tile([128, NP], F32)
        bv_sb = const.tile([1, FL], BF16)

        def emit_const_dma():
            nc.gpsimd.dma_start(out=trimask, in_=mask_d[:, :])
            nc.gpsimd.dma_start(out=bk_sb, in_=bk_d.rearrange("(i p) -> p i", p=128))
            nc.gpsimd.dma_start(out=bq_sb, in_=bq_d.rearrange("(i p) -> p i", p=128))
            nc.gpsimd.dma_start(out=bv_sb, in_=bv_d.rearrange("(o f) -> o f", o=1))

        wp_all = wpool.tile([128, NP, C], BF16, tag="wp")
        wp_t = [wp_all[:, p, :] for p in range(NP)]

        def emit_wp_dma():
            nc.gpsimd.dma_start(
                out=wp_all, in_=wp_d.rearrange("(p q) c -> q p c", q=128)
            )

        # ---- persistent activations ----
        kt_sb = [
            big.tile([128, T], BF16, tag="ktq", bufs=2 * NP, name=f"ktp{p}")
            for p in range(NP)
        ]
        qt_sb = [
            big.tile([128, T], BF16, tag="ktq", bufs=2 * NP, name=f"qtp{p}")
            for p in range(NP)
        ]
        v_all = big.tile([128, NT, HL, D + 1], BF16, tag="v", bufs=1)
        nc.gpsimd.memset(v_all[:, :, :, D : D + 1], 1.0)
        otn = [
            big.tile([128, T], BF16, tag="otn", bufs=NP, name=f"otn{p}")
            for p in range(NP)
        ]

        # ---- filler queue: closures emitted into PE-stall slots ----
        filler = deque()
        pstate = {"credit": 0.0, "rate": 1.0}

        def pump_pace(slots):
            # spread the queued filler evenly over the chunk's pump slots
            pstate["rate"] = len(filler) / max(1, slots)
            pstate["credit"] = 0.0

        def pump():
            pstate["credit"] += pstate["rate"]
            k = int(pstate["credit"])
            pstate["credit"] -= k
            for _ in range(k):
                if not filler:
                    return
                filler.popleft()()

        def pump_all():
            while filler:
                filler.popleft()()

        # ---- per-n-block projection work ----
        def nb_closures(n):
            xt = xtp.tile([128, NCT, 512], BF16, tag="xt", name=f"xt{n}")
            items = []

            def tr(ct):
                psT = gpp.tile([128, 512], BF16, tag="gp", name=f"trp{n}_{ct}")
                for tt in range(4):
                    nc.tensor.transpose(
                        psT[:, ts(tt, 128)],
                        xb_tiles[n][:, tt, ts(ct, 128)],
                        ident,
                    )
                nc.vector.tensor_copy(out=xt[:, ct, :], in_=psT)

            for ct in range(NCT):
                items.append(lambda ct=ct: tr(ct))

            def kq(m):
                src = wk_t if m < NP else wq_t
                mi = m % NP
                dest = kt_sb[mi] if m < NP else qt_sb[mi]
                bias = (bk_sb if m < NP else bq_sb)[:, mi : mi + 1]
                ps = gpp.tile([128, 512], F32, tag="gp", name=f"kqp{n}_{m}")
                for ct in range(NCT):
                    nc.tensor.matmul(
                        out=ps,
                        lhsT=src[ct][:, ts(mi, 128)],
                        rhs=xt[:, ct, :],
                        start=(ct == 0),
                        stop=(ct == NCT - 1),
                    )
                nc.vector.tensor_scalar_add(
                    out=dest[:, ts(n, 512)], in0=ps, scalar1=bias
                )

            # Q pairs first: the next chunk's first S matmuls need Q columns
            for m in (NP, NP + 1, NP + 2, 0, 1, 2):
                items.append(lambda m=m: kq(m))

            def vv(tt):
                t = 4 * n + tt
                ps = gpp.tile([128, 512], F32, tag="gp", name=f"vp{t}")
                for ct in range(NCT):
                    nc.tensor.matmul(
                        out=ps[:, 0:FL],
                        lhsT=xt[:, ct, ts(tt, 128)],
                        rhs=wv_t[ct],
                        start=(ct == 0),
                        stop=False,
                    )
                nc.tensor.matmul(
                    out=ps[:, 0:FL],
                    lhsT=ones_sb,
                    rhs=bv_sb,
                    start=False,
                    stop=True,
                )
                nc.vector.tensor_copy(
                    out=v_all[:, t, :, 0:D],
                    in_=ps[:, 0:FL].rearrange("p (h d) -> p h d", h=HL),
                )

            for tt in range(4):
                items.append(lambda tt=tt: vv(tt))
            return items

        # ---- projection (filler) ----
        def proj_closures(qi):
            y_sb = ypool.tile([128, C], F32, tag="y", name=f"y{qi}")
            items = []

            def half(hf):
                ps = gpp.tile([128, 512], F32, tag="gp", name=f"fp{qi}_{hf}")
                for p in range(NP):
                    nc.tensor.matmul(
                        out=ps[:, 0:FL],
                        lhsT=otn[p][:, ts(qi, 128)],
                        rhs=wp_t[p][:, ts(hf, FL)],
                        start=(p == 0),
                        stop=(p == NP - 1),
                    )
                if hf == 1 and qi < 8:
                    nc.scalar.copy(out=y_sb[:, ts(hf, FL)], in_=ps[:, 0:FL])
                else:
                    nc.vector.tensor_copy(out=y_sb[:, ts(hf, FL)], in_=ps[:, 0:FL])

            items.append(lambda: half(0))
            items.append(lambda: half(1))
            items.append(
                lambda: nc.sync.dma_start(out=y_d[ts(qi, 128), :], in_=y_sb)
            )
            return items

        # ---- attention chunk j (512 queries, all heads) ----
        def emit_chunk(j):
            q0 = 512 * j
            niter = 2 * j + 2  # k-tile pairs
            pump_pace(NP * (2 * niter + 1))
            for p in range(NP):
                hs = (2 * p, 2 * p + 1)
                S = {}
                OT = {}
                for h in hs:
                    S[h] = spool.tile(
                        [128, 1024], F32, tag="s", name=f"s{h}_{j}"
                    )
                    OT[h] = otps.tile(
                        [128, 512], F32, tag="ot", name=f"ot{h}_{j}"
                    )
                for i in range(niter):
                    kts = (2 * i, 2 * i + 1)
                    sA = max(0, 128 * kts[0] - q0)
                    for h in hs:
                        row0 = 64 * (h % 2)
                        kt_ap = kt_sb[p][row0 : row0 + 64, :]
                        qt_ap = qt_sb[p][row0 : row0 + 64, :]
                        for idx, kt in enumerate(kts):
                            c0 = 128 * kt
                            diag = c0 >= q0
                            s0 = max(q0, c0)
                            w = q0 + 512 - s0
                            o = 512 * idx + s0 - q0
                            nc.tensor.matmul(
                                out=S[h][:, o : o + w],
                                lhsT=kt_ap[:, ts(kt, 128)],
                                rhs=qt_ap[:, s0 : s0 + w],
                                start=True,
                                stop=not diag,
                                tile_position=(row0, 0),
                                skip_group_check=True,
                            )
                            if diag:
                                nc.tensor.matmul(
                                    out=S[h][:, o : o + 128],
                                    lhsT=ident,
                                    rhs=trimask,
                                    start=False,
                                    stop=True,
                                    skip_group_check=True,
                                )
                    pump()
                    pbs = {}
                    for h in hs:
                        pb = ppool.tile(
                            [128, 1024], BF16, tag="pb", name=f"pb{h}_{i}_{j}"
                        )
                        nc.scalar.activation(
                            out=pb[:, sA:1024],
                            in_=S[h][:, sA:1024],
                            func=mybir.ActivationFunctionType.Exp,
                            scale=float(D) ** -0.5,
                        )
                        pbs[h] = pb
                    for h in hs:
                        pb = pbs[h]
                        for idx, kt in enumerate(kts):
                            s = max(0, 128 * kt - q0)
                            nc.tensor.matmul(
                                out=OT[h][0 : D + 1, s:512],
                                lhsT=v_all[:, kt, h, :],
                                rhs=pb[:, 512 * idx + s : 512 * idx + 512],
                                start=(i == 0 and idx == 0),
                                stop=(i == niter - 1 and idx == 1),
                                skip_group_check=True,
                            )
                    pump()
                # normalize: r = 1/l broadcast over partitions
                for h in hs:
                    row0 = 64 * (h % 2)
                    lv = smal.tile([1, 512], F32, tag="lv", name=f"lv{h}_{j}")
                    nc.vector.tensor_copy(out=lv, in_=OT[h][D : D + 1, :])
                    rv = smal.tile([1, 512], F32, tag="rv", name=f"rv{h}_{j}")
                    nc.vector.reciprocal_approx_fast(out=rv, in_=lv)
                    rb = smal.tile([64, 512], F32, tag="rb", name=f"rb{h}_{j}")
                    nc.gpsimd.partition_broadcast(rb, rv)
                    nc.vector.tensor_mul(
                        otn[p][row0 : row0 + 64, ts(j, 512)], OT[h][0:D, :], rb
                    )
                    if DEBUG and p == 0:
                        nc.gpsimd.dma_start(
                            out=dbg_l[h : h + 1, ts(j, 512)], in_=lv
                        )
                pump()

        # ---- main schedule ----
        emit_x_dma(0)
        emit_w_dma()
        emit_const_dma()
        emit_x_dma(1)
        emit_wp_dma()
        for it in nb_closures(0):
            it()
        for j in range(NB):
            if j + 1 < NB:
                if j + 2 < NB:
                    emit_x_dma(j + 2)
                filler.extend(nb_closures(j + 1))
            if j >= 1:
                # proj tile qi needs otn chunk qi//4 (ready at chunk qi//4+1)
                for qi in range(4 * (j - 1), 4 * j):
                    filler.extend(proj_closures(qi))
            emit_chunk(j)
            pump_all()
        for qi in range(12, NT):
            for it in proj_closures(qi):
                it()
        if DEBUG:
            nc.gpsimd.dma_start(
                out=dbg_v[:, :], in_=v_all.rearrange("p a b c -> p (a b c)")
            )
            nc.gpsimd.dma_start(out=dbg_kt[:, :], in_=kt_sb[0])
            nc.gpsimd.dma_start(out=dbg_qt[:, :], in_=qt_sb[0])
            nc.gpsimd.dma_start(out=dbg_otn[:, :], in_=otn[0])

    nc.compile()
    return nc


_NC = None


def _get_nc():
    global _NC
    if _NC is None:
        _NC = build_nc()
    return _NC


def make_in_maps(x, W_kqv, b_kqv, W_proj):
    ki = np.arange(128)[:, None]
    qi = np.arange(128)[None, :]
    mask = np.where(ki <= qi, 0.0, MASK_NEG).astype(np.float32)
    in_maps = []
    for core in range(8):
        b = core // 2
        h0 = (core % 2) * HL * D  # feature offset of this core's head group
        in_maps.append(
            {
                "x": np.ascontiguousarray(x[b]),
                "wk": np.ascontiguousarray(W_kqv[:, h0 : h0 + FL]),
                "wq": np.ascontiguousarray(W_kqv[:, C + h0 : C + h0 + FL]),
                "wv": np.ascontiguousarray(W_kqv[:, 2 * C + h0 : 2 * C + h0 + FL]),
                "wp": np.ascontiguousarray(W_proj[h0 : h0 + FL, :]),
                "bk": np.ascontiguousarray(b_kqv[h0 : h0 + FL]),
                "bq": np.ascontiguousarray(b_kqv[C + h0 : C + h0 + FL]),
                "bv": np.ascontiguousarray(b_kqv[2 * C + h0 : 2 * C + h0 + FL]),
                "mask": mask,
            }
        )
    return in_maps


def _combine(results, b_proj):
    y = np.empty((B, T, C), dtype=np.float32)
    for b in range(B):
        y[b] = results[2 * b]["y"] + results[2 * b + 1]["y"] + b_proj[None, :]
    return y


def kernel(x, W_kqv, b_kqv, W_proj, b_proj, **run_kwargs):
    x = np.asarray(x, dtype=np.float32)
    W_kqv = np.asarray(W_kqv, dtype=np.float32)
    b_kqv = np.asarray(b_kqv, dtype=np.float32)
    W_proj = np.asarray(W_proj, dtype=np.float32)
    b_proj = np.asarray(b_proj, dtype=np.float32)

    nc = _get_nc()
    in_maps = make_in_maps(x, W_kqv, b_kqv, W_proj)
    res = run_bass_kernel_spmd(nc, in_maps, core_ids=list(range(8)), **run_kwargs)
    out = _combine(res.results, b_proj)
    kernel.last_result = res
    return out


# revision 19
# speedup vs baseline: 1.9974x; 1.0629x over previous
"""Causal self-attention (B=4, T=2048, C=768, H=12) on 8 TRN2 NeuronCores.

Sharding: DP=4 over batch x TP=2 over heads (6 heads per core).

Single software-pipelined stream per core, ordered so the PE never idles
(keeps the HAM clock-gate warm at 2.4 GHz):

  n-block 0 kqv -> chunk0 attn (+nb1 kqv filler) -> chunk1 attn (+nb2)
  -> chunk2 attn (+nb3) -> chunk3 attn (+proj filler) -> proj tail

Per n-block (512 tokens): X tiles cast-DMA'd to bf16, transposed on the
PE (identity matmul, 4 per PSUM bank, one DVE copy out), then K^T/Q^T
pair tiles [128, T] (bias via DVE tensor_scalar_add) and V natural
[128, (h, 65)] with a ones column so P~@V' also yields the softmax
denominator l as row 64.

Attention per 512-query chunk: two head-chains in flight; S^T for TWO
k-tiles side by side in a [128,1024] PSUM (ragged causal spans), causal
mask added on the diagonal blocks by an extra PE matmul (identity x
trimask accumulate), one exp (ACT, scale=1/8, fp32->bf16) per k-tile
pair, then P~@V' accumulation. Normalize: r = 1/l via DVE reciprocal
straight off the PSUM l-row, gpsimd partition-broadcast, DVE multiply.

Output projection per 128-token tile interleaved as filler; host sums
the two TP partials per batch and adds b_proj.
"""

import sys

sys.path.insert(0, "/opt/trn_rl_repo")

from collections import deque
from contextlib import ExitStack

import numpy as np

import concourse.bass as bass
import concourse.tile as tile
from concourse import bacc
from concourse import mybir
from concourse.bass import ts
from concourse.bass_utils import run_bass_kernel_spmd
from concourse.masks import make_identity

F32 = mybir.dt.float32
BF16 = mybir.dt.bfloat16

B, T, C = 4, 2048, 768
H, D = 12, 64
HL = 6          # heads per core
FL = HL * D     # 384 local feature dim
NCT = C // 128  # 6 contraction tiles
NT = T // 128   # 16 token tiles
NB = T // 512   # 4 n-blocks / query chunks
NP = HL // 2    # 3 head pairs

MASK_NEG = -30000.0

DEBUG = False  # set True (before build) to add intermediate dumps


def build_nc():
    nc = bacc.Bacc()
    x_d = nc.declare_dram_parameter("x", [T, C], F32, isOutput=False)
    wk_d = nc.declare_dram_parameter("wk", [C, FL], F32, isOutput=False)
    wq_d = nc.declare_dram_parameter("wq", [C, FL], F32, isOutput=False)
    wv_d = nc.declare_dram_parameter("wv", [C, FL], F32, isOutput=False)
    wp_d = nc.declare_dram_parameter("wp", [FL, C], F32, isOutput=False)
    bk_d = nc.declare_dram_parameter("bk", [FL], F32, isOutput=False)
    bq_d = nc.declare_dram_parameter("bq", [FL], F32, isOutput=False)
    bv_d = nc.declare_dram_parameter("bv", [FL], F32, isOutput=False)
    mask_d = nc.declare_dram_parameter("mask", [128, 128], F32, isOutput=False)
    y_d = nc.declare_dram_parameter("y", [T, C], F32, isOutput=True)
    if DEBUG:
        dbg_v = nc.declare_dram_parameter(
            "dbg_v", [128, NT * HL * (D + 1)], F32, isOutput=True
        )
        dbg_kt = nc.declare_dram_parameter("dbg_kt", [128, T], F32, isOutput=True)
        dbg_qt = nc.declare_dram_parameter("dbg_qt", [128, T], F32, isOutput=True)
        dbg_otn = nc.declare_dram_parameter("dbg_otn", [128, T], F32, isOutput=True)
        dbg_l = nc.declare_dram_parameter("dbg_l", [2, T], F32, isOutput=True)

    with tile.TileContext(nc) as tc, ExitStack() as ctx:
        const = ctx.enter_context(tc.tile_pool(name="const", bufs=1))
        wpool = ctx.enter_context(tc.tile_pool(name="wpool", bufs=1))
        big = ctx.enter_context(tc.tile_pool(name="big", bufs=1))
        xbp = ctx.enter_context(tc.tile_pool(name="xbp", bufs=3))
        xtp = ctx.enter_context(tc.tile_pool(name="xtp", bufs=2))
        ppool = ctx.enter_context(tc.tile_pool(name="ppool", bufs=4))
        smal = ctx.enter_context(tc.tile_pool(name="smal", bufs=2))
        ypool = ctx.enter_context(tc.tile_pool(name="ypool", bufs=2))
        spool = ctx.enter_context(tc.tile_pool(name="spool", bufs=2, space="PSUM"))
        otps = ctx.enter_context(tc.tile_pool(name="otps", bufs=2, space="PSUM"))
        gpp = ctx.enter_context(tc.tile_pool(name="gpp", bufs=2, space="PSUM"))

        # ---- x n-block 0 first: the PE pipeline starts on it ----
        xb_tiles = {}

        def emit_x_dma(n):
            xb = xbp.tile([128, 4, C], BF16, tag="xb", name=f"xb{n}")
            nc.gpsimd.dma_start(
                out=xb,
                in_=x_d.rearrange("(n t p) c -> p n t c", n=NB, t=4)[:, n, :, :],
            )
            xb_tiles[n] = xb

        # ---- constants / weights: cast-DMA straight to bf16, one DMA each ----
        ident = const.tile([128, 128], BF16)
        make_identity(nc, ident)
        wk_all = wpool.tile([128, NCT, FL], BF16, tag="wk")
        wq_all = wpool.tile([128, NCT, FL], BF16, tag="wq")
        wv_all = wpool.tile([128, NCT, FL], BF16, tag="wv")

        def emit_w_dma():
            for w_all, w_d in ((wk_all, wk_d), (wq_all, wq_d), (wv_all, wv_d)):
                nc.gpsimd.dma_start(
                    out=w_all, in_=w_d.rearrange("(ct p) f -> p ct f", p=128)
                )

        wk_t = [wk_all[:, ct, :] for ct in range(NCT)]
        wq_t = [wq_all[:, ct, :] for ct in range(NCT)]
        wv_t = [wv_all[:, ct, :] for ct in range(NCT)]
        # trimask[k, q] = 0 where k <= q (causal-valid), else MASK_NEG
        trimask = const.tile([128, 128], BF16)
        ones_sb = const.tile([1, 128], BF16)
        nc.vector.memset(ones_sb, 1.0)
        bk_sb = const.tile([128, NP], F32)
        bq_sb = const.tile([128, NP], F32)
        bv_sb = const.tile([1, FL], BF16)

        def emit_const_dma():
            nc.gpsimd.dma_start(out=trimask, in_=mask_d[:, :])
            nc.gpsimd.dma_start(out=bk_sb, in_=bk_d.rearrange("(i p) -> p i", p=128))
            nc.gpsimd.dma_start(out=bq_sb, in_=bq_d.rearrange("(i p) -> p i", p=128))
            nc.gpsimd.dma_start(out=bv_sb, in_=bv_d.rearrange("(o f) -> o f", o=1))

        wp_all = wpool.tile([128, NP, C], BF16, tag="wp")
        wp_t = [wp_all[:, p, :] for p in range(NP)]

        def emit_wp_dma():
            nc.gpsimd.dma_start(
                out=wp_all, in_=wp_d.rearrange("(p q) c -> q p c", q=128)
            )

        # ---- persistent activations ----
        kt_sb = [
            big.tile([128, T], BF16, tag="ktq", bufs=2 * NP, name=f"ktp{p}")
            for p in range(NP)
        ]
        qt_sb = [
            big.tile([128, T], BF16, tag="ktq", bufs=2 * NP, name=f"qtp{p}")
            for p in range(NP)
        ]
        v_all = big.tile([128, NT, HL, D + 1], BF16, tag="v", bufs=1)
        nc.gpsimd.memset(v_all[:, :, :, D : D + 1], 1.0)
        otn = [
            big.tile([128, T], BF16, tag="otn", bufs=NP, name=f"otn{p}")
            for p in range(NP)
        ]

        # ---- filler queue: closures emitted into PE-stall slots ----
        filler = deque()
        pstate = {"credit": 0.0, "rate": 1.0}

        def pump_pace(slots):
            # spread the queued filler evenly over the chunk's pump slots
            pstate["rate"] = len(filler) / max(1, slots)
            pstate["credit"] = 0.0

        def pump():
            pstate["credit"] += pstate["rate"]
            k = int(pstate["credit"])
            pstate["credit"] -= k
            for _ in range(k):
                if not filler:
                    return
                filler.popleft()()

        def pump_all():
            while filler:
                filler.popleft()()

        # ---- per-n-block projection work ----
        def nb_closures(n):
            xt = xtp.tile([128, NCT, 512], BF16, tag="xt", name=f"xt{n}")
            items = []

            def tr(ct):
                psT = gpp.tile([128, 512], BF16, tag="gp", name=f"trp{n}_{ct}")
                for tt in range(4):
                    nc.tensor.transpose(
                        psT[:, ts(tt, 128)],
                        xb_tiles[n][:, tt, ts(ct, 128)],
                        ident,
                    )
                nc.vector.tensor_copy(out=xt[:, ct, :], in_=psT)

            for ct in range(NCT):
                items.append(lambda ct=ct: tr(ct))

            def kq(m):
                src = wk_t if m < NP else wq_t
                mi = m % NP
                dest = kt_sb[mi] if m < NP else qt_sb[mi]
                bias = (bk_sb if m < NP else bq_sb)[:, mi : mi + 1]
                ps = gpp.tile([128, 512], F32, tag="gp", name=f"kqp{n}_{m}")
                for ct in range(NCT):
                    nc.tensor.matmul(
                        out=ps,
                        lhsT=src[ct][:, ts(mi, 128)],
                        rhs=xt[:, ct, :],
                        start=(ct == 0),
                        stop=(ct == NCT - 1),
                    )
                nc.vector.tensor_scalar_add(
                    out=dest[:, ts(n, 512)], in0=ps, scalar1=bias
                )

            # q0,k0,q1,k1,...: pair p's attention unblocks after 2 adds
            for m in (NP, 0, NP + 1, 1, NP + 2, 2):
                items.append(lambda m=m: kq(m))

            def vv(tt):
                t = 4 * n + tt
                ps = gpp.tile([128, 512], F32, tag="gp", name=f"vp{t}")
                for ct in range(NCT):
                    nc.tensor.matmul(
                        out=ps[:, 0:FL],
                        lhsT=xt[:, ct, ts(tt, 128)],
                        rhs=wv_t[ct],
                        start=(ct == 0),
                        stop=False,
                    )
                nc.tensor.matmul(
                    out=ps[:, 0:FL],
                    lhsT=ones_sb,
                    rhs=bv_sb,
                    start=False,
                    stop=True,
                )
                nc.vector.tensor_copy(
                    out=v_all[:, t, :, 0:D],
                    in_=ps[:, 0:FL].rearrange("p (h d) -> p h d", h=HL),
                )

            for tt in range(4):
                items.append(lambda tt=tt: vv(tt))
            return items

        # ---- projection (filler) ----
        def proj_closures(qi):
            y_sb = ypool.tile([128, C], F32, tag="y", name=f"y{qi}")
            items = []

            def half(hf):
                ps = gpp.tile([128, 512], F32, tag="gp", name=f"fp{qi}_{hf}")
                for p in range(NP):
                    nc.tensor.matmul(
                        out=ps[:, 0:FL],
                        lhsT=otn[p][:, ts(qi, 128)],
                        rhs=wp_t[p][:, ts(hf, FL)],
                        start=(p == 0),
                        stop=(p == NP - 1),
                    )
                if hf == 1 and qi < 8:
                    nc.scalar.copy(out=y_sb[:, ts(hf, FL)], in_=ps[:, 0:FL])
                else:
                    nc.vector.tensor_copy(out=y_sb[:, ts(hf, FL)], in_=ps[:, 0:FL])

            items.append(lambda: half(0))
            items.append(lambda: half(1))
            items.append(
                lambda: nc.sync.dma_start(out=y_d[ts(qi, 128), :], in_=y_sb)
            )
            return items

        # ---- attention chunk j (512 queries, all heads) ----
        def emit_chunk(j):
            q0 = 512 * j
            niter = 2 * j + 2  # k-tile pairs
            pump_pace(NP * (2 * niter + 1))
            for p in range(NP):
                hs = (2 * p, 2 * p + 1)
                S = {}
                OT = {}
                for h in hs:
                    S[h] = spool.tile(
                        [128, 1024], F32, tag="s", name=f"s{h}_{j}"
                    )
                    OT[h] = otps.tile(
                        [128, 512], F32, tag="ot", name=f"ot{h}_{j}"
                    )
                for i in range(niter):
                    kts = (2 * i, 2 * i + 1)
                    sA = max(0, 128 * kts[0] - q0)
                    masks = []
                    for h in hs:
                        row0 = 64 * (h % 2)
                        kt_ap = kt_sb[p][row0 : row0 + 64, :]
                        qt_ap = qt_sb[p][row0 : row0 + 64, :]
                        for idx, kt in enumerate(kts):
                            c0 = 128 * kt
                            diag = c0 >= q0
                            s0 = max(q0, c0)
                            w = q0 + 512 - s0
                            o = 512 * idx + s0 - q0
                            nc.tensor.matmul(
                                out=S[h][:, o : o + w],
                                lhsT=kt_ap[:, ts(kt, 128)],
                                rhs=qt_ap[:, s0 : s0 + w],
                                start=True,
                                stop=not diag,
                                tile_position=(row0, 0),
                                skip_group_check=True,
                            )
                            if diag:
                                masks.append((h, o))
                    # one ident weight-load amortized over all diag blocks
                    for h, o in masks:
                        nc.tensor.matmul(
                            out=S[h][:, o : o + 128],
                            lhsT=ident,
                            rhs=trimask,
                            start=False,
                            stop=True,
                            skip_group_check=True,
                        )
                    pump()
                    pbs = {}
                    for h in hs:
                        pb = ppool.tile(
                            [128, 1024], BF16, tag="pb", name=f"pb{h}_{i}_{j}"
                        )
                        nc.scalar.activation(
                            out=pb[:, sA:1024],
                            in_=S[h][:, sA:1024],
                            func=mybir.ActivationFunctionType.Exp,
                            scale=float(D) ** -0.5,
                        )
                        pbs[h] = pb
                    for h in hs:
                        pb = pbs[h]
                        for idx, kt in enumerate(kts):
                            s = max(0, 128 * kt - q0)
                            nc.tensor.matmul(
                                out=OT[h][0 : D + 1, s:512],
                                lhsT=v_all[:, kt, h, :],
                                rhs=pb[:, 512 * idx + s : 512 * idx + 512],
                                start=(i == 0 and idx == 0),
                                stop=(i == niter - 1 and idx == 1),
                                skip_group_check=True,
                            )
                    pump()
                # normalize: r = 1/l broadcast over partitions
                for h in hs:
                    row0 = 64 * (h % 2)
                    lv = smal.tile([1, 512], F32, tag="lv", name=f"lv{h}_{j}")
                    nc.vector.tensor_copy(out=lv, in_=OT[h][D : D + 1, :])
                    rv = smal.tile([1, 512], F32, tag="rv", name=f"rv{h}_{j}")
                    nc.vector.reciprocal_approx_fast(out=rv, in_=lv)
                    rb = smal.tile([64, 512], F32, tag="rb", name=f"rb{h}_{j}")
                    nc.gpsimd.partition_broadcast(rb, rv)
                    nc.vector.tensor_mul(
                        otn[p][row0 : row0 + 64, ts(j, 512)], OT[h][0:D, :], rb
                    )
                    if DEBUG and p == 0:
                        nc.gpsimd.dma_start(
                            out=dbg_l[h : h + 1, ts(j, 512)], in_=lv
                        )
                pump()

        # ---- main schedule ----
        emit_x_dma(0)
        emit_w_dma()
        emit_const_dma()
        emit_x_dma(1)
        emit_wp_dma()
        for it in nb_closures(0):
            it()
        for j in range(NB):
            if j + 1 < NB:
                if j + 2 < NB:
                    emit_x_dma(j + 2)
                filler.extend(nb_closures(j + 1))
            # proj tile qi needs otn chunk qi//4 (ready at chunk qi//4+1);
            # weighted toward chunk3 where the PE needs filler under the
            # ACT-bound exp drain
            proj_sched = {1: range(0, 2), 2: range(2, 5), 3: range(5, 12)}
            for qi in proj_sched.get(j, ()):
                filler.extend(proj_closures(qi))
            emit_chunk(j)
            pump_all()
        for qi in range(12, NT):
            for it in proj_closures(qi):
                it()
        if DEBUG:
            nc.gpsimd.dma_start(
                out=dbg_v[:, :], in_=v_all.rearrange("p a b c -> p (a b c)")
            )
            nc.gpsimd.dma_start(out=dbg_kt[:, :], in_=kt_sb[0])
            nc.gpsimd.dma_start(out=dbg_qt[:, :], in_=qt_sb[0])
            nc.gpsimd.dma_start(out=dbg_otn[:, :], in_=otn[0])

    nc.compile()
    return nc


_NC = None


def _get_nc():
    global _NC
    if _NC is None:
        _NC = build_nc()
    return _NC


def make_in_maps(x, W_kqv, b_kqv, W_proj):
    ki = np.arange(128)[:, None]
    qi = np.arange(128)[None, :]
    mask = np.where(ki <= qi, 0.0, MASK_NEG).astype(np.float32)
    in_maps = []
    for core in range(8):
        b = core // 2
        h0 = (core % 2) * HL * D  # feature offset of this core's head group
        in_maps.append(
            {
                "x": np.ascontiguousarray(x[b]),
                "wk": np.ascontiguousarray(W_kqv[:, h0 : h0 + FL]),
                "wq": np.ascontiguousarray(W_kqv[:, C + h0 : C + h0 + FL]),
                "wv": np.ascontiguousarray(W_kqv[:, 2 * C + h0 : 2 * C + h0 + FL]),
                "wp": np.ascontiguousarray(W_proj[h0 : h0 + FL, :]),
                "bk": np.ascontiguousarray(b_kqv[h0 : h0 + FL]),
                "bq": np.ascontiguousarray(b_kqv[C + h0 : C + h0 + FL]),
                "bv": np.ascontiguousarray(b_kqv[2 * C + h0 : 2 * C + h0 + FL]),
                "mask": mask,
            }
        )
    return in_maps


def _combine(results, b_proj):
    y = np.empty((B, T, C), dtype=np.float32)
    for b in range(B):
        y[b] = results[2 * b]["y"] + results[2 * b + 1]["y"] + b_proj[None, :]
    return y


def kernel(x, W_kqv, b_kqv, W_proj, b_proj, **run_kwargs):
    x = np.asarray(x, dtype=np.float32)
    W_kqv = np.asarray(W_kqv, dtype=np.float32)
    b_kqv = np.asarray(b_kqv, dtype=np.float32)
    W_proj = np.asarray(W_proj, dtype=np.float32)
    b_proj = np.asarray(b_proj, dtype=np.float32)

    nc = _get_nc()
    in_maps = make_in_maps(x, W_kqv, b_kqv, W_proj)
    res = run_bass_kernel_spmd(nc, in_maps, core_ids=list(range(8)), **run_kwargs)
    out = _combine(res.results, b_proj)
    kernel.last_result = res
    return out


# revision 20
# speedup vs baseline: 2.0024x; 1.0025x over previous
"""Causal self-attention (B=4, T=2048, C=768, H=12) on 8 TRN2 NeuronCores.

Sharding: DP=4 over batch x TP=2 over heads (6 heads per core).

Single software-pipelined stream per core, ordered so the PE never idles
(keeps the HAM clock-gate warm at 2.4 GHz):

  n-block 0 kqv -> chunk0 attn (+nb1 kqv filler) -> chunk1 attn (+nb2)
  -> chunk2 attn (+nb3) -> chunk3 attn (+proj filler) -> proj tail

Per n-block (512 tokens): X tiles cast-DMA'd to bf16, transposed on the
PE (identity matmul, 4 per PSUM bank, one DVE copy out), then K^T/Q^T
pair tiles [128, T] (bias via DVE tensor_scalar_add) and V natural
[128, (h, 65)] with a ones column so P~@V' also yields the softmax
denominator l as row 64.

Attention per 512-query chunk: two head-chains in flight; S^T for TWO
k-tiles side by side in a [128,1024] PSUM (ragged causal spans), causal
mask added on the diagonal blocks by an extra PE matmul (identity x
trimask accumulate), one exp (ACT, scale=1/8, fp32->bf16) per k-tile
pair, then P~@V' accumulation. Normalize: r = 1/l via DVE reciprocal
straight off the PSUM l-row, gpsimd partition-broadcast, DVE multiply.

Output projection per 128-token tile interleaved as filler; host sums
the two TP partials per batch and adds b_proj.
"""

import sys

sys.path.insert(0, "/opt/trn_rl_repo")

from collections import deque
from contextlib import ExitStack

import numpy as np

import concourse.bass as bass
import concourse.tile as tile
from concourse import bacc
from concourse import mybir
from concourse.bass import ts
from concourse.bass_utils import run_bass_kernel_spmd
from concourse.masks import make_identity

F32 = mybir.dt.float32
BF16 = mybir.dt.bfloat16

B, T, C = 4, 2048, 768
H, D = 12, 64
HL = 6          # heads per core
FL = HL * D     # 384 local feature dim
NCT = C // 128  # 6 contraction tiles
NT = T // 128   # 16 token tiles
NB = T // 512   # 4 n-blocks / query chunks
NP = HL // 2    # 3 head pairs

MASK_NEG = -30000.0

DEBUG = False  # set True (before build) to add intermediate dumps


def build_nc():
    nc = bacc.Bacc()
    x_d = nc.declare_dram_parameter("x", [T, C], F32, isOutput=False)
    wk_d = nc.declare_dram_parameter("wk", [C, FL], F32, isOutput=False)
    wq_d = nc.declare_dram_parameter("wq", [C, FL], F32, isOutput=False)
    wv_d = nc.declare_dram_parameter("wv", [C, FL], F32, isOutput=False)
    wp_d = nc.declare_dram_parameter("wp", [FL, C], F32, isOutput=False)
    bk_d = nc.declare_dram_parameter("bk", [FL], F32, isOutput=False)
    bq_d = nc.declare_dram_parameter("bq", [FL], F32, isOutput=False)
    bv_d = nc.declare_dram_parameter("bv", [FL], F32, isOutput=False)
    mask_d = nc.declare_dram_parameter("mask", [128, 128], F32, isOutput=False)
    y_d = nc.declare_dram_parameter("y", [T, C], F32, isOutput=True)
    if DEBUG:
        dbg_v = nc.declare_dram_parameter(
            "dbg_v", [128, NT * HL * (D + 1)], F32, isOutput=True
        )
        dbg_kt = nc.declare_dram_parameter("dbg_kt", [128, T], F32, isOutput=True)
        dbg_qt = nc.declare_dram_parameter("dbg_qt", [128, T], F32, isOutput=True)
        dbg_otn = nc.declare_dram_parameter("dbg_otn", [128, T], F32, isOutput=True)
        dbg_l = nc.declare_dram_parameter("dbg_l", [2, T], F32, isOutput=True)

    with tile.TileContext(nc) as tc, ExitStack() as ctx:
        const = ctx.enter_context(tc.tile_pool(name="const", bufs=1))
        wpool = ctx.enter_context(tc.tile_pool(name="wpool", bufs=1))
        big = ctx.enter_context(tc.tile_pool(name="big", bufs=1))
        xbp = ctx.enter_context(tc.tile_pool(name="xbp", bufs=3))
        xtp = ctx.enter_context(tc.tile_pool(name="xtp", bufs=2))
        ppool = ctx.enter_context(tc.tile_pool(name="ppool", bufs=4))
        smal = ctx.enter_context(tc.tile_pool(name="smal", bufs=2))
        ypool = ctx.enter_context(tc.tile_pool(name="ypool", bufs=2))
        spool = ctx.enter_context(tc.tile_pool(name="spool", bufs=2, space="PSUM"))
        otps = ctx.enter_context(tc.tile_pool(name="otps", bufs=2, space="PSUM"))
        gpp = ctx.enter_context(tc.tile_pool(name="gpp", bufs=2, space="PSUM"))

        # ---- x n-block 0 first: the PE pipeline starts on it ----
        xb_tiles = {}

        def emit_x_dma(n):
            xb = xbp.tile([128, 4, C], BF16, tag="xb", name=f"xb{n}")
            nc.gpsimd.dma_start(
                out=xb,
                in_=x_d.rearrange("(n t p) c -> p n t c", n=NB, t=4)[:, n, :, :],
            )
            xb_tiles[n] = xb

        # ---- constants / weights: cast-DMA straight to bf16, one DMA each ----
        ident = const.tile([128, 128], BF16)
        make_identity(nc, ident)
        wk_all = wpool.tile([128, NCT, FL], BF16, tag="wk")
        wq_all = wpool.tile([128, NCT, FL], BF16, tag="wq")
        wv_all = wpool.tile([128, NCT, FL], BF16, tag="wv")

        def emit_w_dma():
            for w_all, w_d in ((wk_all, wk_d), (wq_all, wq_d), (wv_all, wv_d)):
                nc.gpsimd.dma_start(
                    out=w_all, in_=w_d.rearrange("(ct p) f -> p ct f", p=128)
                )

        wk_t = [wk_all[:, ct, :] for ct in range(NCT)]
        wq_t = [wq_all[:, ct, :] for ct in range(NCT)]
        wv_t = [wv_all[:, ct, :] for ct in range(NCT)]
        # trimask[k, q] = 0 where k <= q (causal-valid), else MASK_NEG
        trimask = const.tile([128, 128], BF16)
        ones_sb = const.tile([1, 128], BF16)
        nc.vector.memset(ones_sb, 1.0)
        bk_sb = const.tile([128, NP], F32)
        bq_sb = const.tile([128, NP], F32)
        bv_sb = const.tile([1, FL], BF16)

        def emit_const_dma():
            nc.gpsimd.dma_start(out=trimask, in_=mask_d[:, :])
            nc.gpsimd.dma_start(out=bk_sb, in_=bk_d.rearrange("(i p) -> p i", p=128))
            nc.gpsimd.dma_start(out=bq_sb, in_=bq_d.rearrange("(i p) -> p i", p=128))
            nc.gpsimd.dma_start(out=bv_sb, in_=bv_d.rearrange("(o f) -> o f", o=1))

        wp_all = wpool.tile([128, NP, C], BF16, tag="wp")
        wp_t = [wp_all[:, p, :] for p in range(NP)]

        def emit_wp_dma():
            nc.gpsimd.dma_start(
                out=wp_all, in_=wp_d.rearrange("(p q) c -> q p c", q=128)
            )

        # ---- persistent activations ----
        kt_sb = [
            big.tile([128, T], BF16, tag="ktq", bufs=2 * NP, name=f"ktp{p}")
            for p in range(NP)
        ]
        qt_sb = [
            big.tile([128, T], BF16, tag="ktq", bufs=2 * NP, name=f"qtp{p}")
            for p in range(NP)
        ]
        v_all = big.tile([128, NT, HL, D + 1], BF16, tag="v", bufs=1)
        nc.gpsimd.memset(v_all[:, :, :, D : D + 1], 1.0)
        otn = [
            big.tile([128, T], BF16, tag="otn", bufs=NP, name=f"otn{p}")
            for p in range(NP)
        ]

        # ---- filler queue: closures emitted into PE-stall slots ----
        filler = deque()
        pstate = {"credit": 0.0, "rate": 1.0}

        def pump_pace(slots):
            # spread the queued filler evenly over the chunk's pump slots
            pstate["rate"] = len(filler) / max(1, slots)
            pstate["credit"] = 0.0

        def pump():
            pstate["credit"] += pstate["rate"]
            k = int(pstate["credit"])
            pstate["credit"] -= k
            for _ in range(k):
                if not filler:
                    return
                filler.popleft()()

        def pump_all():
            while filler:
                filler.popleft()()

        # ---- per-n-block projection work ----
        def nb_closures(n):
            xt = xtp.tile([128, NCT, 512], BF16, tag="xt", name=f"xt{n}")
            items = []

            def tr(ct):
                psT = gpp.tile([128, 512], BF16, tag="gp", name=f"trp{n}_{ct}")
                for tt in range(4):
                    nc.tensor.transpose(
                        psT[:, ts(tt, 128)],
                        xb_tiles[n][:, tt, ts(ct, 128)],
                        ident,
                    )
                nc.vector.tensor_copy(out=xt[:, ct, :], in_=psT)

            for ct in range(NCT):
                items.append(lambda ct=ct: tr(ct))

            def kq(m):
                src = wk_t if m < NP else wq_t
                mi = m % NP
                dest = kt_sb[mi] if m < NP else qt_sb[mi]
                bias = (bk_sb if m < NP else bq_sb)[:, mi : mi + 1]
                ps = gpp.tile([128, 512], F32, tag="gp", name=f"kqp{n}_{m}")
                for ct in range(NCT):
                    nc.tensor.matmul(
                        out=ps,
                        lhsT=src[ct][:, ts(mi, 128)],
                        rhs=xt[:, ct, :],
                        start=(ct == 0),
                        stop=(ct == NCT - 1),
                    )
                nc.vector.tensor_scalar_add(
                    out=dest[:, ts(n, 512)], in0=ps, scalar1=bias
                )

            # q0,k0,q1,k1,...: pair p's attention unblocks after 2 adds
            for m in (NP, 0, NP + 1, 1, NP + 2, 2):
                items.append(lambda m=m: kq(m))

            def vv(tt):
                t = 4 * n + tt
                ps = gpp.tile([128, 512], F32, tag="gp", name=f"vp{t}")
                for ct in range(NCT):
                    nc.tensor.matmul(
                        out=ps[:, 0:FL],
                        lhsT=xt[:, ct, ts(tt, 128)],
                        rhs=wv_t[ct],
                        start=(ct == 0),
                        stop=False,
                    )
                nc.tensor.matmul(
                    out=ps[:, 0:FL],
                    lhsT=ones_sb,
                    rhs=bv_sb,
                    start=False,
                    stop=True,
                )
                nc.vector.tensor_copy(
                    out=v_all[:, t, :, 0:D],
                    in_=ps[:, 0:FL].rearrange("p (h d) -> p h d", h=HL),
                )

            for tt in range(4):
                items.append(lambda tt=tt: vv(tt))
            return items

        # ---- projection (filler) ----
        def proj_closures(qi):
            y_sb = ypool.tile([128, C], F32, tag="y", name=f"y{qi}")
            items = []

            def half(hf):
                ps = gpp.tile([128, 512], F32, tag="gp", name=f"fp{qi}_{hf}")
                for p in range(NP):
                    nc.tensor.matmul(
                        out=ps[:, 0:FL],
                        lhsT=otn[p][:, ts(qi, 128)],
                        rhs=wp_t[p][:, ts(hf, FL)],
                        start=(p == 0),
                        stop=(p == NP - 1),
                    )
                if hf == 1 and qi < 2:
                    nc.scalar.copy(out=y_sb[:, ts(hf, FL)], in_=ps[:, 0:FL])
                else:
                    nc.vector.tensor_copy(out=y_sb[:, ts(hf, FL)], in_=ps[:, 0:FL])

            items.append(lambda: half(0))
            items.append(lambda: half(1))
            items.append(
                lambda: nc.sync.dma_start(out=y_d[ts(qi, 128), :], in_=y_sb)
            )
            return items

        # ---- attention chunk j (512 queries, all heads) ----
        def emit_chunk(j):
            q0 = 512 * j
            niter = 2 * j + 2  # k-tile pairs
            pump_pace(NP * (2 * niter + 1))
            for p in range(NP):
                hs = (2 * p, 2 * p + 1)
                S = {}
                OT = {}
                for h in hs:
                    S[h] = spool.tile(
                        [128, 1024], F32, tag="s", name=f"s{h}_{j}"
                    )
                    OT[h] = otps.tile(
                        [128, 512], F32, tag="ot", name=f"ot{h}_{j}"
                    )
                for i in range(niter):
                    kts = (2 * i, 2 * i + 1)
                    sA = max(0, 128 * kts[0] - q0)
                    masks = []
                    for h in hs:
                        row0 = 64 * (h % 2)
                        kt_ap = kt_sb[p][row0 : row0 + 64, :]
                        qt_ap = qt_sb[p][row0 : row0 + 64, :]
                        for idx, kt in enumerate(kts):
                            c0 = 128 * kt
                            diag = c0 >= q0
                            s0 = max(q0, c0)
                            w = q0 + 512 - s0
                            o = 512 * idx + s0 - q0
                            nc.tensor.matmul(
                                out=S[h][:, o : o + w],
                                lhsT=kt_ap[:, ts(kt, 128)],
                                rhs=qt_ap[:, s0 : s0 + w],
                                start=True,
                                stop=not diag,
                                tile_position=(row0, 0),
                                skip_group_check=True,
                            )
                            if diag:
                                masks.append((h, o))
                    # one ident weight-load amortized over all diag blocks
                    for h, o in masks:
                        nc.tensor.matmul(
                            out=S[h][:, o : o + 128],
                            lhsT=ident,
                            rhs=trimask,
                            start=False,
                            stop=True,
                            skip_group_check=True,
                        )
                    pump()
                    pbs = {}
                    for h in hs:
                        pb = ppool.tile(
                            [128, 1024], BF16, tag="pb", name=f"pb{h}_{i}_{j}"
                        )
                        nc.scalar.activation(
                            out=pb[:, sA:1024],
                            in_=S[h][:, sA:1024],
                            func=mybir.ActivationFunctionType.Exp,
                            scale=float(D) ** -0.5,
                        )
                        pbs[h] = pb
                    for h in hs:
                        pb = pbs[h]
                        for idx, kt in enumerate(kts):
                            s = max(0, 128 * kt - q0)
                            nc.tensor.matmul(
                                out=OT[h][0 : D + 1, s:512],
                                lhsT=v_all[:, kt, h, :],
                                rhs=pb[:, 512 * idx + s : 512 * idx + 512],
                                start=(i == 0 and idx == 0),
                                stop=(i == niter - 1 and idx == 1),
                                skip_group_check=True,
                            )
                    pump()
                # normalize: r = 1/l broadcast over partitions
                for h in hs:
                    row0 = 64 * (h % 2)
                    lv = smal.tile([1, 512], F32, tag="lv", name=f"lv{h}_{j}")
                    nc.vector.tensor_copy(out=lv, in_=OT[h][D : D + 1, :])
                    rv = smal.tile([1, 512], F32, tag="rv", name=f"rv{h}_{j}")
                    nc.vector.reciprocal_approx_fast(out=rv, in_=lv)
                    rb = smal.tile([64, 512], F32, tag="rb", name=f"rb{h}_{j}")
                    nc.gpsimd.partition_broadcast(rb, rv)
                    nc.vector.tensor_mul(
                        otn[p][row0 : row0 + 64, ts(j, 512)], OT[h][0:D, :], rb
                    )
                    if DEBUG and p == 0:
                        nc.gpsimd.dma_start(
                            out=dbg_l[h : h + 1, ts(j, 512)], in_=lv
                        )
                pump()

        # ---- main schedule ----
        emit_x_dma(0)
        emit_w_dma()
        emit_const_dma()
        emit_x_dma(1)
        emit_wp_dma()
        for it in nb_closures(0):
            it()
        for j in range(NB):
            if j + 1 < NB:
                if j + 2 < NB:
                    emit_x_dma(j + 2)
                filler.extend(nb_closures(j + 1))
            # proj tile qi needs otn chunk qi//4 (ready at chunk qi//4+1);
            # weighted toward chunk3 where the PE needs filler under the
            # ACT-bound exp drain
            proj_sched = {2: range(0, 2), 3: range(2, 12)}
            for qi in proj_sched.get(j, ()):
                filler.extend(proj_closures(qi))
            emit_chunk(j)
            pump_all()
        for qi in range(12, NT):
            for it in proj_closures(qi):
                it()
        if DEBUG:
            nc.gpsimd.dma_start(
                out=dbg_v[:, :], in_=v_all.rearrange("p a b c -> p (a b c)")
            )
            nc.gpsimd.dma_start(out=dbg_kt[:, :], in_=kt_sb[0])
            nc.gpsimd.dma_start(out=dbg_qt[:, :], in_=qt_sb[0])
            nc.gpsimd.dma_start(out=dbg_otn[:, :], in_=otn[0])

    nc.compile()
    return nc


_NC = None


def _get_nc():
    global _NC
    if _NC is None:
        _NC = build_nc()
    return _NC


def make_in_maps(x, W_kqv, b_kqv, W_proj):
    ki = np.arange(128)[:, None]
    qi = np.arange(128)[None, :]
    mask = np.where(ki <= qi, 0.0, MASK_NEG).astype(np.float32)
    in_maps = []
    for core in range(8):
        b = core // 2
        h0 = (core % 2) * HL * D  # feature offset of this core's head group
        in_maps.append(
            {
                "x": np.ascontiguousarray(x[b]),
                "wk": np.ascontiguousarray(W_kqv[:, h0 : h0 + FL]),
                "wq": np.ascontiguousarray(W_kqv[:, C + h0 : C + h0 + FL]),
                "wv": np.ascontiguousarray(W_kqv[:, 2 * C + h0 : 2 * C + h0 + FL]),
                "wp": np.ascontiguousarray(W_proj[h0 : h0 + FL, :]),
                "bk": np.ascontiguousarray(b_kqv[h0 : h0 + FL]),
                "bq": np.ascontiguousarray(b_kqv[C + h0 : C + h0 + FL]),
                "bv": np.ascontiguousarray(b_kqv[2 * C + h0 : 2 * C + h0 + FL]),
                "mask": mask,
            }
        )
    return in_maps


def _combine(results, b_proj):
    y = np.empty((B, T, C), dtype=np.float32)
    for b in range(B):
        y[b] = results[2 * b]["y"] + results[2 * b + 1]["y"] + b_proj[None, :]
    return y


def kernel(x, W_kqv, b_kqv, W_proj, b_proj, **run_kwargs):
    x = np.asarray(x, dtype=np.float32)
    W_kqv = np.asarray(W_kqv, dtype=np.float32)
    b_kqv = np.asarray(b_kqv, dtype=np.float32)
    W_proj = np.asarray(W_proj, dtype=np.float32)
    b_proj = np.asarray(b_proj, dtype=np.float32)

    nc = _get_nc()
    in_maps = make_in_maps(x, W_kqv, b_kqv, W_proj)
    res = run_bass_kernel_spmd(nc, in_maps, core_ids=list(range(8)), **run_kwargs)
    out = _combine(res.results, b_proj)
    kernel.last_result = res
    return out


# revision 22
# speedup vs baseline: 2.0351x; 1.0163x over previous
"""Causal self-attention (B=4, T=2048, C=768, H=12) on 8 TRN2 NeuronCores.

Sharding: DP=4 over batch x TP=2 over heads (6 heads per core).

Single software-pipelined stream per core, ordered so the PE never idles
(keeps the HAM clock-gate warm at 2.4 GHz):

  n-block 0 kqv -> chunk0 attn (+nb1 kqv filler) -> chunk1 attn (+nb2)
  -> chunk2 attn (+nb3) -> chunk3 attn (+proj filler) -> proj tail

Per n-block (512 tokens): X tiles cast-DMA'd to bf16, transposed on the
PE (identity matmul, 4 per PSUM bank, one DVE copy out), then K^T/Q^T
pair tiles [128, T] (bias via DVE tensor_scalar_add) and V natural
[128, (h, 65)] with a ones column so P~@V' also yields the softmax
denominator l as row 64.

Attention per 512-query chunk: two head-chains in flight; S^T for TWO
k-tiles side by side in a [128,1024] PSUM (ragged causal spans), causal
mask added on the diagonal blocks by an extra PE matmul (identity x
trimask accumulate), one exp (ACT, scale=1/8, fp32->bf16) per k-tile
pair, then P~@V' accumulation. Normalize: r = 1/l via DVE reciprocal
straight off the PSUM l-row, gpsimd partition-broadcast, DVE multiply.

Output projection per 128-token tile interleaved as filler; host sums
the two TP partials per batch and adds b_proj.
"""

import sys

sys.path.insert(0, "/opt/trn_rl_repo")

from collections import deque
from contextlib import ExitStack

import numpy as np

import concourse.bass as bass
import concourse.tile as tile
from concourse import bacc
from concourse import mybir
from concourse.bass import ts
from concourse.bass_utils import run_bass_kernel_spmd
from concourse.masks import make_identity

F32 = mybir.dt.float32
BF16 = mybir.dt.bfloat16

B, T, C = 4, 2048, 768
H, D = 12, 64
HL = 6          # heads per core
FL = HL * D     # 384 local feature dim
NCT = C // 128  # 6 contraction tiles
NT = T // 128   # 16 token tiles
NB = T // 512   # 4 n-blocks / query chunks
NP = HL // 2    # 3 head pairs

MASK_NEG = -30000.0

DEBUG = False  # set True (before build) to add intermediate dumps


def build_nc():
    nc = bacc.Bacc()
    x_d = nc.declare_dram_parameter("x", [T, C], F32, isOutput=False)
    wk_d = nc.declare_dram_parameter("wk", [C, FL], F32, isOutput=False)
    wq_d = nc.declare_dram_parameter("wq", [C, FL], F32, isOutput=False)
    wv_d = nc.declare_dram_parameter("wv", [C, FL], F32, isOutput=False)
    wp_d = nc.declare_dram_parameter("wp", [FL, C], F32, isOutput=False)
    bk_d = nc.declare_dram_parameter("bk", [FL], F32, isOutput=False)
    bq_d = nc.declare_dram_parameter("bq", [FL], F32, isOutput=False)
    bv_d = nc.declare_dram_parameter("bv", [FL], F32, isOutput=False)
    mask_d = nc.declare_dram_parameter("mask", [128, 128], F32, isOutput=False)
    y_d = nc.declare_dram_parameter("y", [T, C], F32, isOutput=True)
    if DEBUG:
        dbg_v = nc.declare_dram_parameter(
            "dbg_v", [128, NT * HL * (D + 1)], F32, isOutput=True
        )
        dbg_kt = nc.declare_dram_parameter("dbg_kt", [128, T], F32, isOutput=True)
        dbg_qt = nc.declare_dram_parameter("dbg_qt", [128, T], F32, isOutput=True)
        dbg_otn = nc.declare_dram_parameter("dbg_otn", [128, T], F32, isOutput=True)
        dbg_l = nc.declare_dram_parameter("dbg_l", [2, T], F32, isOutput=True)

    with tile.TileContext(nc) as tc, ExitStack() as ctx:
        const = ctx.enter_context(tc.tile_pool(name="const", bufs=1))
        wpool = ctx.enter_context(tc.tile_pool(name="wpool", bufs=1))
        big = ctx.enter_context(tc.tile_pool(name="big", bufs=1))
        xbp = ctx.enter_context(tc.tile_pool(name="xbp", bufs=3))
        xtp = ctx.enter_context(tc.tile_pool(name="xtp", bufs=2))
        ppool = ctx.enter_context(tc.tile_pool(name="ppool", bufs=4))
        smal = ctx.enter_context(tc.tile_pool(name="smal", bufs=2))
        ypool = ctx.enter_context(tc.tile_pool(name="ypool", bufs=2))
        spool = ctx.enter_context(tc.tile_pool(name="spool", bufs=2, space="PSUM"))
        otps = ctx.enter_context(tc.tile_pool(name="otps", bufs=2, space="PSUM"))
        gpp = ctx.enter_context(tc.tile_pool(name="gpp", bufs=2, space="PSUM"))

        # ---- x n-block 0 first: the PE pipeline starts on it ----
        xb_tiles = {}

        def emit_x_dma(n):
            xb = xbp.tile([128, 4, C], BF16, tag="xb", name=f"xb{n}")
            nc.gpsimd.dma_start(
                out=xb,
                in_=x_d.rearrange("(n t p) c -> p n t c", n=NB, t=4)[:, n, :, :],
            )
            xb_tiles[n] = xb

        # ---- constants / weights: cast-DMA straight to bf16, one DMA each ----
        ident = const.tile([128, 128], BF16)
        make_identity(nc, ident)
        wk_all = wpool.tile([128, NCT, FL], BF16, tag="wk")
        wq_all = wpool.tile([128, NCT, FL], BF16, tag="wq")
        wv_all = wpool.tile([128, NCT, FL], BF16, tag="wv")

        def emit_w_dma():
            for w_all, w_d in ((wk_all, wk_d), (wq_all, wq_d), (wv_all, wv_d)):
                nc.gpsimd.dma_start(
                    out=w_all, in_=w_d.rearrange("(ct p) f -> p ct f", p=128)
                )

        wk_t = [wk_all[:, ct, :] for ct in range(NCT)]
        wq_t = [wq_all[:, ct, :] for ct in range(NCT)]
        wv_t = [wv_all[:, ct, :] for ct in range(NCT)]
        # trimask[k, q] = 0 where k <= q (causal-valid), else MASK_NEG
        trimask = const.tile([128, 128], BF16)
        ones_sb = const.tile([1, 128], BF16)
        nc.vector.memset(ones_sb, 1.0)
        bk_sb = const.tile([128, NP], F32)
        bq_sb = const.tile([128, NP], F32)
        bv_sb = const.tile([1, FL], BF16)

        def emit_const_dma():
            nc.gpsimd.dma_start(out=trimask, in_=mask_d[:, :])
            nc.gpsimd.dma_start(out=bk_sb, in_=bk_d.rearrange("(i p) -> p i", p=128))
            nc.gpsimd.dma_start(out=bq_sb, in_=bq_d.rearrange("(i p) -> p i", p=128))
            nc.gpsimd.dma_start(out=bv_sb, in_=bv_d.rearrange("(o f) -> o f", o=1))

        wp_all = wpool.tile([128, NP, C], BF16, tag="wp")
        wp_t = [wp_all[:, p, :] for p in range(NP)]

        def emit_wp_dma():
            nc.gpsimd.dma_start(
                out=wp_all, in_=wp_d.rearrange("(p q) c -> q p c", q=128)
            )

        # ---- persistent activations ----
        kt_sb = [
            big.tile([128, T], BF16, tag="ktq", bufs=2 * NP, name=f"ktp{p}")
            for p in range(NP)
        ]
        qt_sb = [
            big.tile([128, T], BF16, tag="ktq", bufs=2 * NP, name=f"qtp{p}")
            for p in range(NP)
        ]
        v_all = big.tile([128, NT, HL, D + 1], BF16, tag="v", bufs=1)
        nc.gpsimd.memset(v_all[:, :, :, D : D + 1], 1.0)
        otn = [
            big.tile([128, T], BF16, tag="otn", bufs=NP, name=f"otn{p}")
            for p in range(NP)
        ]

        # ---- filler queue: closures emitted into PE-stall slots ----
        filler = deque()
        pstate = {"credit": 0.0, "rate": 1.0}

        def pump_pace(slots):
            # spread the queued filler evenly over the chunk's pump slots
            pstate["rate"] = len(filler) / max(1, slots)
            pstate["credit"] = 0.0

        def pump():
            pstate["credit"] += pstate["rate"]
            k = int(pstate["credit"])
            pstate["credit"] -= k
            for _ in range(k):
                if not filler:
                    return
                filler.popleft()()

        def pump_all():
            while filler:
                filler.popleft()()

        # ---- per-n-block projection work ----
        def nb_closures(n):
            xt = xtp.tile([128, NCT, 512], BF16, tag="xt", name=f"xt{n}")
            items = []

            def tr(ct):
                psT = gpp.tile([128, 512], BF16, tag="gp", name=f"trp{n}_{ct}")
                for tt in range(4):
                    nc.tensor.transpose(
                        psT[:, ts(tt, 128)],
                        xb_tiles[n][:, tt, ts(ct, 128)],
                        ident,
                    )
                nc.vector.tensor_copy(out=xt[:, ct, :], in_=psT)

            for ct in range(NCT):
                items.append(lambda ct=ct: tr(ct))

            def kq(m):
                src = wk_t if m < NP else wq_t
                mi = m % NP
                dest = kt_sb[mi] if m < NP else qt_sb[mi]
                bias = (bk_sb if m < NP else bq_sb)[:, mi : mi + 1]
                ps = gpp.tile([128, 512], F32, tag="gp", name=f"kqp{n}_{m}")
                for ct in range(NCT):
                    nc.tensor.matmul(
                        out=ps,
                        lhsT=src[ct][:, ts(mi, 128)],
                        rhs=xt[:, ct, :],
                        start=(ct == 0),
                        stop=(ct == NCT - 1),
                    )
                nc.vector.tensor_scalar_add(
                    out=dest[:, ts(n, 512)], in0=ps, scalar1=bias
                )

            # q0,k0,q1,k1,...: pair p's attention unblocks after 2 adds
            for m in (NP, 0, NP + 1, 1, NP + 2, 2):
                items.append(lambda m=m: kq(m))

            def vv(tt):
                t = 4 * n + tt
                ps = gpp.tile([128, 512], F32, tag="gp", name=f"vp{t}")
                for ct in range(NCT):
                    nc.tensor.matmul(
                        out=ps[:, 0:FL],
                        lhsT=xt[:, ct, ts(tt, 128)],
                        rhs=wv_t[ct],
                        start=(ct == 0),
                        stop=False,
                    )
                nc.tensor.matmul(
                    out=ps[:, 0:FL],
                    lhsT=ones_sb,
                    rhs=bv_sb,
                    start=False,
                    stop=True,
                )
                nc.vector.tensor_copy(
                    out=v_all[:, t, :, 0:D],
                    in_=ps[:, 0:FL].rearrange("p (h d) -> p h d", h=HL),
                )

            for tt in range(4):
                items.append(lambda tt=tt: vv(tt))
            return items

        # ---- projection (filler) ----
        def proj_closures(qi):
            y_sb = ypool.tile([128, C], F32, tag="y", name=f"y{qi}")
            items = []

            def half(hf):
                ps = gpp.tile([128, 512], F32, tag="gp", name=f"fp{qi}_{hf}")
                for p in range(NP):
                    nc.tensor.matmul(
                        out=ps[:, 0:FL],
                        lhsT=otn[p][:, ts(qi, 128)],
                        rhs=wp_t[p][:, ts(hf, FL)],
                        start=(p == 0),
                        stop=(p == NP - 1),
                    )
                if hf == 1 and qi < 2:
                    nc.scalar.copy(out=y_sb[:, ts(hf, FL)], in_=ps[:, 0:FL])
                else:
                    nc.vector.tensor_copy(out=y_sb[:, ts(hf, FL)], in_=ps[:, 0:FL])

            items.append(lambda: half(0))
            items.append(lambda: half(1))
            items.append(
                lambda: nc.sync.dma_start(out=y_d[ts(qi, 128), :], in_=y_sb)
            )
            return items

        # ---- attention chunk j (512 queries, all heads) ----
        def emit_chunk(j):
            q0 = 512 * j
            niter = 2 * j + 2  # k-tile pairs
            pump_pace(NP * (2 * niter + 1))
            for p in range(NP):
                hs = (2 * p, 2 * p + 1)
                S = {}
                OT = {}
                for h in hs:
                    S[h] = spool.tile(
                        [128, 1024], F32, tag="s", name=f"s{h}_{j}"
                    )
                    OT[h] = otps.tile(
                        [128, 512], F32, tag="ot", name=f"ot{h}_{j}"
                    )
                for i in range(niter):
                    kts = (2 * i, 2 * i + 1)
                    sA = max(0, 128 * kts[0] - q0)
                    masks = []
                    for h in hs:
                        row0 = 64 * (h % 2)
                        kt_ap = kt_sb[p][row0 : row0 + 64, :]
                        qt_ap = qt_sb[p][row0 : row0 + 64, :]
                        for idx, kt in enumerate(kts):
                            c0 = 128 * kt
                            diag = c0 >= q0
                            s0 = max(q0, c0)
                            w = q0 + 512 - s0
                            o = 512 * idx + s0 - q0
                            nc.tensor.matmul(
                                out=S[h][:, o : o + w],
                                lhsT=kt_ap[:, ts(kt, 128)],
                                rhs=qt_ap[:, s0 : s0 + w],
                                start=True,
                                stop=not diag,
                                tile_position=(row0, 0),
                                skip_group_check=True,
                            )
                            if diag:
                                masks.append((h, o))
                    # one ident weight-load amortized over all diag blocks
                    for h, o in masks:
                        nc.tensor.matmul(
                            out=S[h][:, o : o + 128],
                            lhsT=ident,
                            rhs=trimask,
                            start=False,
                            stop=True,
                            skip_group_check=True,
                        )
                    pump()
                    pbs = {}
                    for h in hs:
                        pb = ppool.tile(
                            [128, 1024], BF16, tag="pb", name=f"pb{h}_{i}_{j}"
                        )
                        nc.scalar.activation(
                            out=pb[:, sA:1024],
                            in_=S[h][:, sA:1024],
                            func=mybir.ActivationFunctionType.Exp,
                            scale=float(D) ** -0.5,
                        )
                        pbs[h] = pb
                    for h in hs:
                        pb = pbs[h]
                        for idx, kt in enumerate(kts):
                            s = max(0, 128 * kt - q0)
                            nc.tensor.matmul(
                                out=OT[h][0 : D + 1, s:512],
                                lhsT=v_all[:, kt, h, :],
                                rhs=pb[:, 512 * idx + s : 512 * idx + 512],
                                start=(i == 0 and idx == 0),
                                stop=(i == niter - 1 and idx == 1),
                                skip_group_check=True,
                            )
                    pump()
                # normalize: r = 1/l broadcast over partitions
                for h in hs:
                    row0 = 64 * (h % 2)
                    lv = smal.tile([1, 512], F32, tag="lv", name=f"lv{h}_{j}")
                    nc.vector.tensor_copy(out=lv, in_=OT[h][D : D + 1, :])
                    rv = smal.tile([1, 512], F32, tag="rv", name=f"rv{h}_{j}")
                    nc.vector.reciprocal_approx_fast(out=rv, in_=lv)
                    rb = smal.tile([64, 512], F32, tag="rb", name=f"rb{h}_{j}")
                    nc.gpsimd.partition_broadcast(rb, rv)
                    nc.vector.tensor_mul(
                        otn[p][row0 : row0 + 64, ts(j, 512)], OT[h][0:D, :], rb
                    )
                    if DEBUG and p == 0:
                        nc.gpsimd.dma_start(
                            out=dbg_l[h : h + 1, ts(j, 512)], in_=lv
                        )
                pump()

        # ---- main schedule ----
        emit_x_dma(0)
        emit_w_dma()
        emit_const_dma()
        emit_x_dma(1)
        emit_wp_dma()
        # warm the PE clock-gate (HAM) with junk matmuls while the first
        # x DMA is in flight: one fully-busy 4096-cycle window un-throttles
        # the PE to 2.4 GHz before real work arrives
        warm = gpp.tile([128, 512], F32, tag="gp", name="warm")
        for _ in range(32):
            nc.tensor.matmul(
                out=warm[:, 0:128],
                lhsT=ident,
                rhs=ident,
                start=True,
                stop=True,
                skip_group_check=True,
            )
        for it in nb_closures(0):
            it()
        for j in range(NB):
            if j + 1 < NB:
                if j + 2 < NB:
                    emit_x_dma(j + 2)
                filler.extend(nb_closures(j + 1))
            # proj tile qi needs otn chunk qi//4 (ready at chunk qi//4+1);
            # weighted toward chunk3 where the PE needs filler under the
            # ACT-bound exp drain
            proj_sched = {2: range(0, 2), 3: range(2, 12)}
            for qi in proj_sched.get(j, ()):
                filler.extend(proj_closures(qi))
            emit_chunk(j)
            pump_all()
        for qi in range(12, NT):
            for it in proj_closures(qi):
                it()
        if DEBUG:
            nc.gpsimd.dma_start(
                out=dbg_v[:, :], in_=v_all.rearrange("p a b c -> p (a b c)")
            )
            nc.gpsimd.dma_start(out=dbg_kt[:, :], in_=kt_sb[0])
            nc.gpsimd.dma_start(out=dbg_qt[:, :], in_=qt_sb[0])
            nc.gpsimd.dma_start(out=dbg_otn[:, :], in_=otn[0])

    nc.compile()
    return nc


_NC = None


def _get_nc():
    global _NC
    if _NC is None:
        _NC = build_nc()
    return _NC


def make_in_maps(x, W_kqv, b_kqv, W_proj):
    ki = np.arange(128)[:, None]
    qi = np.arange(128)[None, :]
    mask = np.where(ki <= qi, 0.0, MASK_NEG).astype(np.float32)
    in_maps = []
    for core in range(8):
        b = core // 2
        h0 = (core % 2) * HL * D  # feature offset of this core's head group
        in_maps.append(
            {
                "x": np.ascontiguousarray(x[b]),
                "wk": np.ascontiguousarray(W_kqv[:, h0 : h0 + FL]),
                "wq": np.ascontiguousarray(W_kqv[:, C + h0 : C + h0 + FL]),
                "wv": np.ascontiguousarray(W_kqv[:, 2 * C + h0 : 2 * C + h0 + FL]),
                "wp": np.ascontiguousarray(W_proj[h0 : h0 + FL, :]),
                "bk": np.ascontiguousarray(b_kqv[h0 : h0 + FL]),
                "bq": np.ascontiguousarray(b_kqv[C + h0 : C + h0 + FL]),
                "bv": np.ascontiguousarray(b_kqv[2 * C + h0 : 2 * C + h0 + FL]),
                "mask": mask,
            }
        )
    return in_maps


def _combine(results, b_proj):
    y = np.empty((B, T, C), dtype=np.float32)
    for b in range(B):
        y[b] = results[2 * b]["y"] + results[2 * b + 1]["y"] + b_proj[None, :]
    return y


def kernel(x, W_kqv, b_kqv, W_proj, b_proj, **run_kwargs):
    x = np.asarray(x, dtype=np.float32)
    W_kqv = np.asarray(W_kqv, dtype=np.float32)
    b_kqv = np.asarray(b_kqv, dtype=np.float32)
    W_proj = np.asarray(W_proj, dtype=np.float32)
    b_proj = np.asarray(b_proj, dtype=np.float32)

    nc = _get_nc()
    in_maps = make_in_maps(x, W_kqv, b_kqv, W_proj)
    res = run_bass_kernel_spmd(nc, in_maps, core_ids=list(range(8)), **run_kwargs)
    out = _combine(res.results, b_proj)
    kernel.last_result = res
    return out


# revision 23
# speedup vs baseline: 2.0736x; 1.0189x over previous
"""Causal self-attention (B=4, T=2048, C=768, H=12) on 8 TRN2 NeuronCores.

Sharding: DP=4 over batch x TP=2 over heads (6 heads per core).

Single software-pipelined stream per core, ordered so the PE never idles
(keeps the HAM clock-gate warm at 2.4 GHz):

  n-block 0 kqv -> chunk0 attn (+nb1 kqv filler) -> chunk1 attn (+nb2)
  -> chunk2 attn (+nb3) -> chunk3 attn (+proj filler) -> proj tail

Per n-block (512 tokens): X tiles cast-DMA'd to bf16, transposed on the
PE (identity matmul, 4 per PSUM bank, one DVE copy out), then K^T/Q^T
pair tiles [128, T] (bias via DVE tensor_scalar_add) and V natural
[128, (h, 65)] with a ones column so P~@V' also yields the softmax
denominator l as row 64.

Attention per 512-query chunk: two head-chains in flight; S^T for TWO
k-tiles side by side in a [128,1024] PSUM (ragged causal spans), causal
mask added on the diagonal blocks by an extra PE matmul (identity x
trimask accumulate), one exp (ACT, scale=1/8, fp32->bf16) per k-tile
pair, then P~@V' accumulation. Normalize: r = 1/l via DVE reciprocal
straight off the PSUM l-row, gpsimd partition-broadcast, DVE multiply.

Output projection per 128-token tile interleaved as filler; host sums
the two TP partials per batch and adds b_proj.
"""

import sys

sys.path.insert(0, "/opt/trn_rl_repo")

from collections import deque
from contextlib import ExitStack

import numpy as np

import concourse.bass as bass
import concourse.tile as tile
from concourse import bacc
from concourse import mybir
from concourse.bass import ts
from concourse.bass_utils import run_bass_kernel_spmd
from concourse.masks import make_identity

F32 = mybir.dt.float32
BF16 = mybir.dt.bfloat16

B, T, C = 4, 2048, 768
H, D = 12, 64
HL = 6          # heads per core
FL = HL * D     # 384 local feature dim
NCT = C // 128  # 6 contraction tiles
NT = T // 128   # 16 token tiles
NB = T // 512   # 4 n-blocks / query chunks
NP = HL // 2    # 3 head pairs

MASK_NEG = -30000.0

DEBUG = False  # set True (before build) to add intermediate dumps


def build_nc():
    nc = bacc.Bacc()
    x_d = nc.declare_dram_parameter("x", [T, C], F32, isOutput=False)
    wk_d = nc.declare_dram_parameter("wk", [C, FL], F32, isOutput=False)
    wq_d = nc.declare_dram_parameter("wq", [C, FL], F32, isOutput=False)
    wv_d = nc.declare_dram_parameter("wv", [C, FL], F32, isOutput=False)
    wp_d = nc.declare_dram_parameter("wp", [FL, C], F32, isOutput=False)
    bk_d = nc.declare_dram_parameter("bk", [FL], F32, isOutput=False)
    bq_d = nc.declare_dram_parameter("bq", [FL], F32, isOutput=False)
    bv_d = nc.declare_dram_parameter("bv", [FL], F32, isOutput=False)
    mask_d = nc.declare_dram_parameter("mask", [128, 128], F32, isOutput=False)
    y_d = nc.declare_dram_parameter("y", [T, C], F32, isOutput=True)
    if DEBUG:
        dbg_v = nc.declare_dram_parameter(
            "dbg_v", [128, NT * HL * (D + 1)], F32, isOutput=True
        )
        dbg_kt = nc.declare_dram_parameter("dbg_kt", [128, T], F32, isOutput=True)
        dbg_qt = nc.declare_dram_parameter("dbg_qt", [128, T], F32, isOutput=True)
        dbg_otn = nc.declare_dram_parameter("dbg_otn", [128, T], F32, isOutput=True)
        dbg_l = nc.declare_dram_parameter("dbg_l", [2, T], F32, isOutput=True)

    with tile.TileContext(nc) as tc, ExitStack() as ctx:
        const = ctx.enter_context(tc.tile_pool(name="const", bufs=1))
        wpool = ctx.enter_context(tc.tile_pool(name="wpool", bufs=1))
        big = ctx.enter_context(tc.tile_pool(name="big", bufs=1))
        xbp = ctx.enter_context(tc.tile_pool(name="xbp", bufs=3))
        xtp = ctx.enter_context(tc.tile_pool(name="xtp", bufs=2))
        ppool = ctx.enter_context(tc.tile_pool(name="ppool", bufs=4))
        smal = ctx.enter_context(tc.tile_pool(name="smal", bufs=2))
        ypool = ctx.enter_context(tc.tile_pool(name="ypool", bufs=2))
        spool = ctx.enter_context(tc.tile_pool(name="spool", bufs=2, space="PSUM"))
        otps = ctx.enter_context(tc.tile_pool(name="otps", bufs=2, space="PSUM"))
        gpp = ctx.enter_context(tc.tile_pool(name="gpp", bufs=2, space="PSUM"))

        # ---- x n-block 0 first: the PE pipeline starts on it ----
        xb_tiles = {}

        def emit_x_dma(n):
            xb = xbp.tile([128, 4, C], BF16, tag="xb", name=f"xb{n}")
            nc.gpsimd.dma_start(
                out=xb,
                in_=x_d.rearrange("(n t p) c -> p n t c", n=NB, t=4)[:, n, :, :],
            )
            xb_tiles[n] = xb

        # ---- constants / weights: cast-DMA straight to bf16, one DMA each ----
        ident = const.tile([128, 128], BF16)
        make_identity(nc, ident)
        wk_all = wpool.tile([128, NCT, FL], BF16, tag="wk")
        wq_all = wpool.tile([128, NCT, FL], BF16, tag="wq")
        wv_all = wpool.tile([128, NCT, FL], BF16, tag="wv")

        def emit_w_dma():
            for w_all, w_d in ((wk_all, wk_d), (wq_all, wq_d), (wv_all, wv_d)):
                nc.gpsimd.dma_start(
                    out=w_all, in_=w_d.rearrange("(ct p) f -> p ct f", p=128)
                )

        wk_t = [wk_all[:, ct, :] for ct in range(NCT)]
        wq_t = [wq_all[:, ct, :] for ct in range(NCT)]
        wv_t = [wv_all[:, ct, :] for ct in range(NCT)]
        # trimask[k, q] = 0 where k <= q (causal-valid), else MASK_NEG
        trimask = const.tile([128, 128], BF16)
        ones_sb = const.tile([1, 128], BF16)
        nc.vector.memset(ones_sb, 1.0)
        bk_sb = const.tile([128, NP], F32)
        bq_sb = const.tile([128, NP], F32)
        bv_sb = const.tile([1, FL], BF16)

        def emit_const_dma():
            nc.gpsimd.dma_start(out=trimask, in_=mask_d[:, :])
            nc.gpsimd.dma_start(out=bk_sb, in_=bk_d.rearrange("(i p) -> p i", p=128))
            nc.gpsimd.dma_start(out=bq_sb, in_=bq_d.rearrange("(i p) -> p i", p=128))
            nc.gpsimd.dma_start(out=bv_sb, in_=bv_d.rearrange("(o f) -> o f", o=1))

        wp_all = wpool.tile([128, NP, C], BF16, tag="wp")
        wp_t = [wp_all[:, p, :] for p in range(NP)]

        def emit_wp_dma():
            nc.gpsimd.dma_start(
                out=wp_all, in_=wp_d.rearrange("(p q) c -> q p c", q=128)
            )

        # ---- persistent activations ----
        kt_sb = [
            big.tile([128, T], BF16, tag="ktq", bufs=2 * NP, name=f"ktp{p}")
            for p in range(NP)
        ]
        qt_sb = [
            big.tile([128, T], BF16, tag="ktq", bufs=2 * NP, name=f"qtp{p}")
            for p in range(NP)
        ]
        v_all = big.tile([128, NT, HL, D + 1], BF16, tag="v", bufs=1)
        nc.gpsimd.memset(v_all[:, :, :, D : D + 1], 1.0)
        otn = [
            big.tile([128, T], BF16, tag="otn", bufs=NP, name=f"otn{p}")
            for p in range(NP)
        ]

        # ---- filler queue: closures emitted into PE-stall slots ----
        filler = deque()
        pstate = {"credit": 0.0, "rate": 1.0}

        def pump_pace(slots):
            # spread the queued filler evenly over the chunk's pump slots
            pstate["rate"] = len(filler) / max(1, slots)
            pstate["credit"] = 0.0

        def pump():
            pstate["credit"] += pstate["rate"]
            k = int(pstate["credit"])
            pstate["credit"] -= k
            for _ in range(k):
                if not filler:
                    return
                filler.popleft()()

        def pump_all():
            while filler:
                filler.popleft()()

        # ---- per-n-block projection work ----
        def nb_closures(n):
            xt = xtp.tile([128, NCT, 512], BF16, tag="xt", name=f"xt{n}")
            items = []
            pcnt = {"i": 0}

            def palloc(dtype, name):
                # nb0 runs before attention: rotate over the idle attention
                # PSUM pools too, so PE doesn't serialize on the 2 gpp banks
                if n != 0:
                    return gpp.tile([128, 512], dtype, tag="gp", name=name)
                r = pcnt["i"] % 3
                pcnt["i"] += 1
                if r == 0:
                    return gpp.tile([128, 512], dtype, tag="gp", name=name)
                if r == 1:
                    return spool.tile([128, 512], dtype, tag="s", name=name)
                return otps.tile([128, 512], dtype, tag="ot", name=name)

            def tr(ct):
                psT = palloc(BF16, f"trp{n}_{ct}")
                for tt in range(4):
                    nc.tensor.transpose(
                        psT[:, ts(tt, 128)],
                        xb_tiles[n][:, tt, ts(ct, 128)],
                        ident,
                    )
                nc.vector.tensor_copy(out=xt[:, ct, :], in_=psT)

            for ct in range(NCT):
                items.append(lambda ct=ct: tr(ct))

            def kq(m):
                src = wk_t if m < NP else wq_t
                mi = m % NP
                dest = kt_sb[mi] if m < NP else qt_sb[mi]
                bias = (bk_sb if m < NP else bq_sb)[:, mi : mi + 1]
                ps = palloc(F32, f"kqp{n}_{m}")
                for ct in range(NCT):
                    nc.tensor.matmul(
                        out=ps,
                        lhsT=src[ct][:, ts(mi, 128)],
                        rhs=xt[:, ct, :],
                        start=(ct == 0),
                        stop=(ct == NCT - 1),
                    )
                nc.vector.tensor_scalar_add(
                    out=dest[:, ts(n, 512)], in0=ps, scalar1=bias
                )

            # q0,k0,q1,k1,...: pair p's attention unblocks after 2 adds
            for m in (NP, 0, NP + 1, 1, NP + 2, 2):
                items.append(lambda m=m: kq(m))

            def vv(tt):
                t = 4 * n + tt
                ps = palloc(F32, f"vp{t}")
                for ct in range(NCT):
                    nc.tensor.matmul(
                        out=ps[:, 0:FL],
                        lhsT=xt[:, ct, ts(tt, 128)],
                        rhs=wv_t[ct],
                        start=(ct == 0),
                        stop=False,
                    )
                nc.tensor.matmul(
                    out=ps[:, 0:FL],
                    lhsT=ones_sb,
                    rhs=bv_sb,
                    start=False,
                    stop=True,
                )
                nc.vector.tensor_copy(
                    out=v_all[:, t, :, 0:D],
                    in_=ps[:, 0:FL].rearrange("p (h d) -> p h d", h=HL),
                )

            for tt in range(4):
                items.append(lambda tt=tt: vv(tt))
            return items

        # ---- projection (filler) ----
        def proj_closures(qi):
            y_sb = ypool.tile([128, C], F32, tag="y", name=f"y{qi}")
            items = []

            def half(hf):
                ps = gpp.tile([128, 512], F32, tag="gp", name=f"fp{qi}_{hf}")
                for p in range(NP):
                    nc.tensor.matmul(
                        out=ps[:, 0:FL],
                        lhsT=otn[p][:, ts(qi, 128)],
                        rhs=wp_t[p][:, ts(hf, FL)],
                        start=(p == 0),
                        stop=(p == NP - 1),
                    )
                if hf == 1 and qi < 2:
                    nc.scalar.copy(out=y_sb[:, ts(hf, FL)], in_=ps[:, 0:FL])
                else:
                    nc.vector.tensor_copy(out=y_sb[:, ts(hf, FL)], in_=ps[:, 0:FL])

            items.append(lambda: half(0))
            items.append(lambda: half(1))
            items.append(
                lambda: nc.sync.dma_start(out=y_d[ts(qi, 128), :], in_=y_sb)
            )
            return items

        # ---- attention chunk j (512 queries, all heads) ----
        def emit_chunk(j):
            q0 = 512 * j
            niter = 2 * j + 2  # k-tile pairs
            pump_pace(NP * (2 * niter + 1))
            for p in range(NP):
                hs = (2 * p, 2 * p + 1)
                S = {}
                OT = {}
                for h in hs:
                    S[h] = spool.tile(
                        [128, 1024], F32, tag="s", name=f"s{h}_{j}"
                    )
                    OT[h] = otps.tile(
                        [128, 512], F32, tag="ot", name=f"ot{h}_{j}"
                    )
                for i in range(niter):
                    kts = (2 * i, 2 * i + 1)
                    sA = max(0, 128 * kts[0] - q0)
                    masks = []
                    for h in hs:
                        row0 = 64 * (h % 2)
                        kt_ap = kt_sb[p][row0 : row0 + 64, :]
                        qt_ap = qt_sb[p][row0 : row0 + 64, :]
                        for idx, kt in enumerate(kts):
                            c0 = 128 * kt
                            diag = c0 >= q0
                            s0 = max(q0, c0)
                            w = q0 + 512 - s0
                            o = 512 * idx + s0 - q0
                            nc.tensor.matmul(
                                out=S[h][:, o : o + w],
                                lhsT=kt_ap[:, ts(kt, 128)],
                                rhs=qt_ap[:, s0 : s0 + w],
                                start=True,
                                stop=not diag,
                                tile_position=(row0, 0),
                                skip_group_check=True,
                            )
                            if diag:
                                masks.append((h, o))
                    # one ident weight-load amortized over all diag blocks
                    for h, o in masks:
                        nc.tensor.matmul(
                            out=S[h][:, o : o + 128],
                            lhsT=ident,
                            rhs=trimask,
                            start=False,
                            stop=True,
                            skip_group_check=True,
                        )
                    pump()
                    pbs = {}
                    for h in hs:
                        pb = ppool.tile(
                            [128, 1024], BF16, tag="pb", name=f"pb{h}_{i}_{j}"
                        )
                        nc.scalar.activation(
                            out=pb[:, sA:1024],
                            in_=S[h][:, sA:1024],
                            func=mybir.ActivationFunctionType.Exp,
                            scale=float(D) ** -0.5,
                        )
                        pbs[h] = pb
                    for h in hs:
                        pb = pbs[h]
                        for idx, kt in enumerate(kts):
                            s = max(0, 128 * kt - q0)
                            nc.tensor.matmul(
                                out=OT[h][0 : D + 1, s:512],
                                lhsT=v_all[:, kt, h, :],
                                rhs=pb[:, 512 * idx + s : 512 * idx + 512],
                                start=(i == 0 and idx == 0),
                                stop=(i == niter - 1 and idx == 1),
                                skip_group_check=True,
                            )
                    pump()
                # normalize: r = 1/l broadcast over partitions
                for h in hs:
                    row0 = 64 * (h % 2)
                    lv = smal.tile([1, 512], F32, tag="lv", name=f"lv{h}_{j}")
                    nc.vector.tensor_copy(out=lv, in_=OT[h][D : D + 1, :])
                    rv = smal.tile([1, 512], F32, tag="rv", name=f"rv{h}_{j}")
                    nc.vector.reciprocal_approx_fast(out=rv, in_=lv)
                    rb = smal.tile([64, 512], F32, tag="rb", name=f"rb{h}_{j}")
                    nc.gpsimd.partition_broadcast(rb, rv)
                    nc.vector.tensor_mul(
                        otn[p][row0 : row0 + 64, ts(j, 512)], OT[h][0:D, :], rb
                    )
                    if DEBUG and p == 0:
                        nc.gpsimd.dma_start(
                            out=dbg_l[h : h + 1, ts(j, 512)], in_=lv
                        )
                pump()

        # ---- main schedule ----
        emit_x_dma(0)
        emit_w_dma()
        emit_const_dma()
        emit_x_dma(1)
        emit_wp_dma()
        # warm the PE clock-gate (HAM) with junk matmuls while the first
        # x DMA is in flight: one fully-busy 4096-cycle window un-throttles
        # the PE to 2.4 GHz before real work arrives
        warm = gpp.tile([128, 512], F32, tag="gp", name="warm")
        for _ in range(32):
            nc.tensor.matmul(
                out=warm[:, 0:128],
                lhsT=ident,
                rhs=ident,
                start=True,
                stop=True,
                skip_group_check=True,
            )
        for it in nb_closures(0):
            it()
        for j in range(NB):
            if j + 1 < NB:
                if j + 2 < NB:
                    emit_x_dma(j + 2)
                filler.extend(nb_closures(j + 1))
            # proj tile qi needs otn chunk qi//4 (ready at chunk qi//4+1);
            # weighted toward chunk3 where the PE needs filler under the
            # ACT-bound exp drain
            proj_sched = {2: range(0, 2), 3: range(2, 12)}
            for qi in proj_sched.get(j, ()):
                filler.extend(proj_closures(qi))
            emit_chunk(j)
            pump_all()
        for qi in range(12, NT):
            for it in proj_closures(qi):
                it()
        if DEBUG:
            nc.gpsimd.dma_start(
                out=dbg_v[:, :], in_=v_all.rearrange("p a b c -> p (a b c)")
            )
            nc.gpsimd.dma_start(out=dbg_kt[:, :], in_=kt_sb[0])
            nc.gpsimd.dma_start(out=dbg_qt[:, :], in_=qt_sb[0])
            nc.gpsimd.dma_start(out=dbg_otn[:, :], in_=otn[0])

    nc.compile()
    return nc


_NC = None


def _get_nc():
    global _NC
    if _NC is None:
        _NC = build_nc()
    return _NC


def make_in_maps(x, W_kqv, b_kqv, W_proj):
    ki = np.arange(128)[:, None]
    qi = np.arange(128)[None, :]
    mask = np.where(ki <= qi, 0.0, MASK_NEG).astype(np.float32)
    in_maps = []
    for core in range(8):
        b = core // 2
        h0 = (core % 2) * HL * D  # feature offset of this core's head group
        in_maps.append(
            {
                "x": np.ascontiguousarray(x[b]),
                "wk": np.ascontiguousarray(W_kqv[:, h0 : h0 + FL]),
                "wq": np.ascontiguousarray(W_kqv[:, C + h0 : C + h0 + FL]),
                "wv": np.ascontiguousarray(W_kqv[:, 2 * C + h0 : 2 * C + h0 + FL]),
                "wp": np.ascontiguousarray(W_proj[h0 : h0 + FL, :]),
                "bk": np.ascontiguousarray(b_kqv[h0 : h0 + FL]),
                "bq": np.ascontiguousarray(b_kqv[C + h0 : C + h0 + FL]),
                "bv": np.ascontiguousarray(b_kqv[2 * C + h0 : 2 * C + h0 + FL]),
                "mask": mask,
            }
        )
    return in_maps


def _combine(results, b_proj):
    y = np.empty((B, T, C), dtype=np.float32)
    for b in range(B):
        y[b] = results[2 * b]["y"] + results[2 * b + 1]["y"] + b_proj[None, :]
    return y


def kernel(x, W_kqv, b_kqv, W_proj, b_proj, **run_kwargs):
    x = np.asarray(x, dtype=np.float32)
    W_kqv = np.asarray(W_kqv, dtype=np.float32)
    b_kqv = np.asarray(b_kqv, dtype=np.float32)
    W_proj = np.asarray(W_proj, dtype=np.float32)
    b_proj = np.asarray(b_proj, dtype=np.float32)

    nc = _get_nc()
    in_maps = make_in_maps(x, W_kqv, b_kqv, W_proj)
    res = run_bass_kernel_spmd(nc, in_maps, core_ids=list(range(8)), **run_kwargs)
    out = _combine(res.results, b_proj)
    kernel.last_result = res
    return out
